# revision 1
# baseline (speedup 1.0000x reference)
"""LocalPatchAttention Trainium2 kernel.

Data-parallel over batch B=8 across 8 NeuronCores (one image per core).
Per-core pipeline (all channel counts hardcoded for the B,Cq,H,W = 8,64,256,256 /
Cv,h,w = 128,64,64 problem):

  - q rows stream in [64ch, 512px] pairs (2 image rows).
  - PE-transpose 128-px chunks -> [128px, 64ch] in PSUM; LayerNorm stats via
    bn_stats/bn_aggr on VectorE (free-dim reduce); normalize with a 2-op
    tensor_scalar ((x-mu)*rsqrt) writing bf16.
  - PE-transpose back to [64ch, 128px]; one matmul with the host-prefolded
    [64,128] matrix A = scale * (g*qW^T) @ K^T gives the attention logits;
    Sigmoid on ScalarE with the folded bias as per-partition bias.
  - x_attn = sig * V via stride-0 broadcast APs (V per 4x4 patch cell), V
    precomputed once per core with the same LN/linear folding.
  - 3x3 conv = 12 PSUM-accumulated matmuls per 4 output rows, output-channel
    dim packed 2 rows deep (M=128); conv bias folded in as a K=1 matmul;
    fp32 residual add with the resident q rows; stream out.
"""

import numpy as np
import ml_dtypes

import concourse.bass as bass
import concourse.bacc as bacc
import concourse.tile as tile
from concourse import mybir
from concourse.bass_utils import run_bass_kernel_spmd

F32 = mybir.dt.float32
BF16 = mybir.dt.bfloat16
AF = mybir.ActivationFunctionType
ALU = mybir.AluOpType
EPS = 1e-5
NPBF16 = ml_dtypes.bfloat16

_CACHE = {}


def _build_nc():
    nc = bacc.Bacc()
    q_d = nc.declare_dram_parameter("q", [64, 65536], F32, isOutput=False)
    v_d = nc.declare_dram_parameter("v", [128, 4096], F32, isOutput=False)
    A_d = nc.declare_dram_parameter("Amat", [64, 128], BF16, isOutput=False)
    cb_d = nc.declare_dram_parameter("cbias", [128, 1], F32, isOutput=False)
    vwf_d = nc.declare_dram_parameter("vwf", [128, 128], BF16, isOutput=False)
    vbp_d = nc.declare_dram_parameter("vbp", [128, 1], F32, isOutput=False)
    cwt_d = nc.declare_dram_parameter("cwt", [128, 1536], BF16, isOutput=False)
    cbb_d = nc.declare_dram_parameter("cbb", [1, 128], BF16, isOutput=False)
    i64_d = nc.declare_dram_parameter("i64", [64, 64], BF16, isOutput=False)
    i128_d = nc.declare_dram_parameter("i128", [128, 128], BF16, isOutput=False)
    out_d = nc.declare_dram_parameter("out", [64, 65536], F32, isOutput=True)

    with tile.TileContext(nc) as tc, \
         tc.tile_pool(name="const", bufs=1) as cpool, \
         tc.tile_pool(name="vwork", bufs=1) as vpool, \
         tc.tile_pool(name="qin", bufs=6) as qin_pool, \
         tc.tile_pool(name="qb", bufs=4) as qb_pool, \
         tc.tile_pool(name="xh", bufs=8) as xh_pool, \
         tc.tile_pool(name="xhT", bufs=3) as xhT_pool, \
         tc.tile_pool(name="sig", bufs=4) as sig_pool, \
         tc.tile_pool(name="srow", bufs=16) as srow_pool, \
         tc.tile_pool(name="stat", bufs=8) as st_pool, \
         tc.tile_pool(name="outp", bufs=3) as out_pool, \
         tc.tile_pool(name="ps_tp", bufs=4, space="PSUM") as ps_tp, \
         tc.tile_pool(name="ps_lg", bufs=2, space="PSUM") as ps_lg, \
         tc.tile_pool(name="ps_cv", bufs=2, space="PSUM") as ps_cv:

        def const_tile(shape, dtype, tag, src):
            t = cpool.tile(shape, dtype, tag=tag)
            nc.sync.dma_start(out=t, in_=src[:, :])
            return t

        A_sb = const_tile([64, 128], BF16, "A", A_d)
        cb_sb = const_tile([128, 1], F32, "cb", cb_d)
        vwf_sb = const_tile([128, 128], BF16, "vwf", vwf_d)
        vbp_sb = const_tile([128, 1], F32, "vbp", vbp_d)
        cwt_sb = const_tile([128, 1536], BF16, "cwt", cwt_d)
        cbb_sb = const_tile([1, 128], BF16, "cbb", cbb_d)
        i64_sb = const_tile([64, 64], BF16, "i64", i64_d)
        i128_sb = const_tile([128, 128], BF16, "i128", i128_d)

        ones512b = cpool.tile([1, 512], BF16, tag="o512")
        nc.vector.memset(ones512b, 1.0)
        ones128f = cpool.tile([128, 1], F32, tag="o128")
        nc.vector.memset(ones128f, 1.0)
        ones1x128 = cpool.tile([1, 128], F32, tag="o1x")
        nc.vector.memset(ones1x128, 1.0)
        zrow = cpool.tile([128, 256], BF16, tag="zr")
        nc.vector.memset(zrow, 0.0)

        # ---------------- V path (once per core) ----------------
        vraw = vpool.tile([128, 4096], F32, tag="vraw")
        vsq = vpool.tile([128, 4096], F32, tag="vsq")
        vhat = vpool.tile([128, 4096], BF16, tag="vhat")
        V_sb = vpool.tile([128, 4096], F32, tag="V")
        for ch in range(8):
            sl = slice(ch * 512, (ch + 1) * 512)
            nc.sync.dma_start(out=vraw[:, sl], in_=v_d[:, sl])
            nc.scalar.activation(vsq[:, sl], vraw[:, sl], AF.Square)
            s_ps = ps_tp.tile([1, 512], F32, tag="t")
            nc.tensor.matmul(s_ps, ones128f, vraw[:, sl], start=True, stop=True)
            sq_ps = ps_tp.tile([1, 512], F32, tag="t")
            nc.tensor.matmul(sq_ps, ones128f, vsq[:, sl], start=True, stop=True)
            mu = st_pool.tile([1, 512], F32, tag="vmu")
            nc.vector.tensor_scalar_mul(mu, s_ps, 1.0 / 128)
            var = st_pool.tile([1, 512], F32, tag="vvar")
            nc.vector.tensor_mul(var, mu, mu)
            msq = st_pool.tile([1, 512], F32, tag="vmsq")
            nc.vector.tensor_scalar(msq, sq_ps, 1.0 / 128, None, ALU.mult)
            nc.vector.tensor_sub(var, msq, var)
            nc.vector.tensor_scalar_add(var, var, EPS)
            rec = st_pool.tile([1, 512], F32, tag="vrec")
            nc.vector.reciprocal(rec, var)
            rr = st_pool.tile([1, 512], F32, tag="vr")
            nc.scalar.activation(rr, rec, AF.Sqrt)
            rb = ps_lg.tile([128, 512], F32, tag="lg")
            nc.tensor.matmul(rb, ones1x128, rr, start=True, stop=True)
            mb = ps_lg.tile([128, 512], F32, tag="lg")
            nc.tensor.matmul(mb, ones1x128, mu, start=True, stop=True)
            tmp = st_pool.tile([128, 512], F32, tag="vtmp")
            nc.vector.tensor_sub(tmp, vraw[:, sl], mb)
            nc.vector.tensor_mul(vhat[:, sl], tmp, rb)
        for ch in range(8):
            sl = slice(ch * 512, (ch + 1) * 512)
            vp = ps_lg.tile([128, 512], F32, tag="lg")
            nc.tensor.matmul(vp, vwf_sb, vhat[:, sl], start=True, stop=True)
            nc.vector.tensor_scalar_add(V_sb[:, sl], vp, vbp_sb[:, 0:1])

        # ---------------- main loop ----------------
        srows = {}
        qins = {}

        def attn_pair(pi):
            y = 2 * pi
            qin = qin_pool.tile([64, 512], F32, tag="qin")
            nc.sync.dma_start(out=qin, in_=q_d[:, y * 256:(y + 2) * 256])
            qins[pi] = qin
            qb = qb_pool.tile([64, 512], BF16, tag="qb")
            nc.scalar.copy(qb, qin)
            xhT_ps = ps_tp.tile([64, 512], F32, tag="t")
            for c in range(4):
                csl = slice(c * 128, (c + 1) * 128)
                t1 = ps_tp.tile([128, 64], F32, tag="t")
                nc.tensor.matmul(t1, qb[:, csl], i64_sb, start=True, stop=True)
                st6 = st_pool.tile([128, 6], F32, tag="st6")
                nc.vector.bn_stats(st6, t1)
                mv = st_pool.tile([128, 2], F32, tag="mv")
                nc.vector.bn_aggr(mv, st6)
                rec = st_pool.tile([128, 1], F32, tag="rec")
                nc.vector.tensor_scalar_add(rec, mv[:, 1:2], EPS)
                nc.vector.reciprocal(rec, rec)
                rr = st_pool.tile([128, 1], F32, tag="rr")
                nc.scalar.activation(rr, rec, AF.Sqrt)
                xh = xh_pool.tile([128, 64], BF16, tag="xh")
                nc.vector.tensor_scalar(xh, t1, mv[:, 0:1], rr,
                                        ALU.subtract, ALU.mult)
                nc.tensor.matmul(xhT_ps[:, csl], xh, i128_sb,
                                 start=True, stop=True)
            xhT = xhT_pool.tile([64, 512], BF16, tag="xhT")
            nc.scalar.copy(xhT, xhT_ps)
            lg = ps_lg.tile([128, 512], F32, tag="lg")
            nc.tensor.matmul(lg, A_sb, xhT, start=True, stop=True)
            sig = sig_pool.tile([128, 512], BF16, tag="sig")
            nc.scalar.activation(sig, lg, AF.Sigmoid, bias=cb_sb[:, 0:1])
            hy = y // 4
            vsl = V_sb[:, hy * 64:(hy + 1) * 64]
            vb_ap = vsl.rearrange("p c -> p c ()").broadcast_to([128, 64, 4])
            for r in range(2):
                srow = srow_pool.tile([128, 256], BF16, tag="srow")
                nc.vector.tensor_mul(
                    srow.rearrange("p (c f) -> p c f", f=4),
                    sig[:, r * 256:(r + 1) * 256].rearrange("p (c f) -> p c f", f=4),
                    vb_ap,
                )
                srows[y + r] = srow

        def conv_block(y0):
            cv = ps_cv.tile([128, 512], F32, tag="cv")
            nc.tensor.matmul(cv, cbb_sb, ones512b, start=True, stop=False)
            for bi, dx in enumerate((1, 0, 2)):
                for ti, t in enumerate((-1, 0, 1, 2)):
                    blk = bi * 4 + ti
                    wt = cwt_sb[:, blk * 128:(blk + 1) * 128]
                    last = (dx == 2 and t == 2)
                    for p in range(2):
                        r = y0 + 2 * p + t
                        rt = srows[r] if 0 <= r <= 255 else zrow
                        base = p * 256
                        if dx == 1:
                            nc.tensor.matmul(cv[:, base:base + 256], wt,
                                             rt[:, 0:256], start=False, stop=last)
                        elif dx == 0:
                            nc.tensor.matmul(cv[:, base + 1:base + 256], wt,
                                             rt[:, 0:255], start=False, stop=last)
                        else:
                            nc.tensor.matmul(cv[:, base:base + 255], wt,
                                             rt[:, 1:256], start=False, stop=last)
            for p in range(2):
                y = y0 + 2 * p
                qin = qins.pop(y // 2)
                ot = out_pool.tile([64, 512], F32, tag="ot")
                nc.vector.tensor_add(ot[:, 0:256], cv[0:64, p * 256:(p + 1) * 256],
                                     qin[:, 0:256])
                nc.vector.tensor_add(ot[:, 256:512], cv[64:128, p * 256:(p + 1) * 256],
                                     qin[:, 256:512])
                nc.sync.dma_start(out=out_d[:, y * 256:(y + 2) * 256], in_=ot)
            for r in list(srows):
                if r < y0 + 1:
                    del srows[r]

        for pi in range(129):
            if pi < 128:
                attn_pair(pi)
            if pi >= 2 and pi % 2 == 0:
                conv_block(2 * pi - 4)

    nc.finalize()
    return nc


def _fold_weights(qW, qb, vW, vb, K, qn_g, qn_b, vn_g, vn_b, cW, cb):
    f = np.float32
    qW, qb, vW, vb, K = f(qW), f(qb), f(vW), f(vb), f(K)
    qn_g, qn_b, vn_g, vn_b, cW, cb = f(qn_g), f(qn_b), f(vn_g), f(vn_b), f(cW), f(cb)
    scale = np.float32(64.0 ** -0.5)
    qWf = qn_g[:, None] * qW.T                      # [c, co]
    bprime = qb + qW @ qn_b                         # [64]
    A = scale * (qWf @ K.T)                         # [64, 128]
    c_b = scale * (K @ bprime)                      # [128]
    vWf = vn_g[:, None] * vW.T                      # [128, 128]
    vbp = vb + vW @ vn_b                            # [128]
    cwt = np.zeros((128, 12, 128), np.float32)
    for bi, dx in enumerate((1, 0, 2)):
        for ti, t in enumerate((-1, 0, 1, 2)):
            blk = bi * 4 + ti
            if 0 <= t + 1 <= 2:
                cwt[:, blk, 0:64] = cW[:, :, t + 1, dx].T
            if 0 <= t <= 2:
                cwt[:, blk, 64:128] = cW[:, :, t, dx].T
    return {
        "Amat": np.ascontiguousarray(A.astype(NPBF16)),
        "cbias": np.ascontiguousarray(c_b.reshape(128, 1)),
        "vwf": np.ascontiguousarray(vWf.astype(NPBF16)),
        "vbp": np.ascontiguousarray(vbp.reshape(128, 1)),
        "cwt": np.ascontiguousarray(cwt.reshape(128, 1536).astype(NPBF16)),
        "cbb": np.ascontiguousarray(np.concatenate([cb, cb]).reshape(1, 128).astype(NPBF16)),
        "i64": np.ascontiguousarray(np.eye(64, dtype=np.float32).astype(NPBF16)),
        "i128": np.ascontiguousarray(np.eye(128, dtype=np.float32).astype(NPBF16)),
    }


def _run(in_maps, trace=False, **kw):
    if "nc" not in _CACHE:
        _CACHE["nc"] = _build_nc()
    return run_bass_kernel_spmd(_CACHE["nc"], in_maps, list(range(8)),
                                trace=trace, **kw)


def kernel(q, v, qW, qb, vW, vb, K, qn_g, qn_b, vn_g, vn_b, cW, cb):
    base = _fold_weights(qW, qb, vW, vb, K, qn_g, qn_b, vn_g, vn_b, cW, cb)
    in_maps = []
    for i in range(8):
        m = dict(base)
        m["q"] = np.ascontiguousarray(np.float32(q[i]).reshape(64, 65536))
        m["v"] = np.ascontiguousarray(np.float32(v[i]).reshape(128, 4096))
        in_maps.append(m)
    res = _run(in_maps)
    outs = [np.asarray(r["out"], np.float32).reshape(64, 256, 256)
            for r in res.results]
    return np.stack(outs)



# revision 17
# speedup vs baseline: 2.2133x; 2.2133x over previous
"""LocalPatchAttention Trainium2 kernel.

Data-parallel over batch B=8 across 8 NeuronCores (one image per core).
Per-core pipeline for B,Cq,H,W = 8,64,256,256 / Cv,h,w = 128,64,64.

Transpose-free LayerNorm-attention formulation:
  logits[v,px] = r_px * (A.T q[:,px] - mu_px * u) + cb
with A = scale*(g*qW.T)@K.T prefolded on host, u = colsum(A),
mu/E[q^2] per pixel computed by PE ones-matmuls on float32r views of the
raw f32 q rows (1 cycle/row, no bf16 copy), and the per-pixel row math
(var, 1/sqrt) batched over 16 row-pairs so its DVE/Act cost amortizes.
r is broadcast across the 128 v-channels by a rank-1 ones-matmul; the
single DVE multiply X = lg * R feeds Sigmoid (bias = folded cb).
x_attn = sig * V uses a pre-replicated bf16 V (V_rep) so the multiply
runs in the DVE fast mode. 3x3 conv = 12 PSUM-accumulated bf16 matmuls
per 4 output rows (2-row-deep output packing), conv bias as a K=1
matmul, and the residual q added by two identity matmuls per row pair
(float32r). conv PSUM is copied once to bf16 SBUF and DMAed out as
bf16 (upcast on host).

Activation usage stays inside {Square, Copy, Sigmoid} plus one Sqrt per
16-pair batch, so act-table reloads drop from ~2/pair to 2/batch.
"""

import numpy as np
import ml_dtypes

import concourse.bass as bass
import concourse.bacc as bacc
import concourse.tile as tile
from concourse import mybir
from concourse.bass_utils import run_bass_kernel_spmd

F32 = mybir.dt.float32
F32R = mybir.dt.float32r
BF16 = mybir.dt.bfloat16
AF = mybir.ActivationFunctionType
ALU = mybir.AluOpType
EPS = 1e-5
NPBF16 = ml_dtypes.bfloat16

_CACHE = {}


def _build_nc():
    nc = bacc.Bacc()
    q_d = nc.declare_dram_parameter("q", [64, 65536], F32R, isOutput=False)
    v_d = nc.declare_dram_parameter("v", [128, 4096], F32, isOutput=False)
    A_d = nc.declare_dram_parameter("Amat", [64, 128], BF16, isOutput=False)
    stx_d = nc.declare_dram_parameter("stx", [128, 768], F32R, isOutput=False)
    negu_d = nc.declare_dram_parameter("negu", [16, 2048], BF16, isOutput=False)
    onesel_d = nc.declare_dram_parameter("onesel", [16, 2048], BF16, isOutput=False)
    ip_d = nc.declare_dram_parameter("ipair", [64, 256], F32R, isOutput=False)
    cb_d = nc.declare_dram_parameter("cbias", [128, 1], F32, isOutput=False)
    vwf_d = nc.declare_dram_parameter("vwf", [128, 128], BF16, isOutput=False)
    vbp_d = nc.declare_dram_parameter("vbp", [128, 1], F32, isOutput=False)
    cwt_d = nc.declare_dram_parameter("cwt", [128, 1536], BF16, isOutput=False)
    cbb_d = nc.declare_dram_parameter("cbb", [128, 1], F32, isOutput=False)
    out_d = nc.declare_dram_parameter("out", [64, 65536], BF16, isOutput=True)

    with tile.TileContext(nc) as tc, \
         tc.tile_pool(name="const", bufs=1) as cpool, \
         tc.tile_pool(name="vwork", bufs=1) as vpool, \
         tc.tile_pool(name="qq", bufs=11) as qq_pool, \
         tc.tile_pool(name="stat", bufs=2) as st_pool, \
         tc.tile_pool(name="xt", bufs=3) as x_pool, \
         tc.tile_pool(name="sig", bufs=3) as sig_pool, \
         tc.tile_pool(name="srow", bufs=12) as srow_pool, \
         tc.tile_pool(name="outp", bufs=3) as out_pool, \
         tc.tile_pool(name="ps_s", bufs=2, space="PSUM") as ps_s, \
         tc.tile_pool(name="ps_lg", bufs=2, space="PSUM") as ps_lg, \
         tc.tile_pool(name="ps_r", bufs=2, space="PSUM") as ps_r, \
         tc.tile_pool(name="ps_cv", bufs=2, space="PSUM") as ps_cv:

        def const_tile(shape, dtype, tag, src):
            t = cpool.tile(shape, dtype, tag=tag)
            nc.sync.dma_start(out=t, in_=src[:, :])
            return t

        A_sb = const_tile([64, 128], BF16, "A", A_d)
        stx_sb = const_tile([128, 768], F32R, "stx", stx_d)
        negu_sb = const_tile([16, 2048], BF16, "negu", negu_d)
        onesel_sb = const_tile([16, 2048], BF16, "onesel", onesel_d)
        ip_sb = const_tile([64, 256], F32R, "ip", ip_d)
        cb_sb = const_tile([128, 1], F32, "cb", cb_d)
        vwf_sb = const_tile([128, 128], BF16, "vwf", vwf_d)
        vbp_sb = const_tile([128, 1], F32, "vbp", vbp_d)
        cwt_sb = const_tile([128, 1536], BF16, "cwt", cwt_d)
        cbb_sb = const_tile([128, 1], F32, "cbb", cbb_d)

        ones1x128b = cpool.tile([1, 128], BF16, tag="o1x")
        nc.vector.memset(ones1x128b, 1.0)
        ones128b = cpool.tile([128, 1], BF16, tag="o128")
        nc.vector.memset(ones128b, 1.0)
        zrow = cpool.tile([128, 256], BF16, tag="zr")
        nc.vector.memset(zrow, 0.0)
        zline = cpool.tile([1, 512], BF16, tag="zl")
        nc.vector.memset(zline, 0.0)
        zcol = cpool.tile([1, 128], BF16, tag="zc")
        nc.vector.memset(zcol, 0.0)
        epsc = cpool.tile([128, 1], F32, tag="eps")
        nc.vector.memset(epsc, EPS)

        # ---------------- V path (once per core) ----------------
        vraw = vpool.tile([128, 4096], F32, tag="vraw")
        vb16 = vpool.tile([128, 4096], BF16, tag="vb16")
        vsqb = vpool.tile([128, 4096], BF16, tag="vsqb")
        vhat = vpool.tile([128, 4096], BF16, tag="vhat")
        V_sb = vpool.tile([128, 4096], F32, tag="V")
        V_rep = vpool.tile([128, 16384], BF16, tag="Vrep")
        for ch in range(8):
            sl = slice(ch * 512, (ch + 1) * 512)
            nc.sync.dma_start(out=vraw[:, sl], in_=v_d[:, sl])
            nc.scalar.copy(vb16[:, sl], vraw[:, sl])
            nc.scalar.activation(vsqb[:, sl], vb16[:, sl], AF.Square)
            st_ps = ps_r.tile([128, 512], F32, tag="R")
            nc.tensor.matmul(st_ps[0:1, :], ones128b, vb16[:, sl],
                             start=True, stop=True)
            nc.tensor.matmul(st_ps[32:33, :], ones128b, vsqb[:, sl],
                             start=True, stop=True)
            mu = st_pool.tile([1, 512], BF16, tag="vmu")
            with nc.allow_low_precision(reason="bf16 LN mean; tol 2e-2"):
                nc.vector.tensor_scalar_mul(mu, st_ps[0:1, :], 1.0 / 128)
            var = st_pool.tile([1, 512], F32, tag="vvar")
            nc.vector.tensor_mul(var, mu, mu)
            msq = st_pool.tile([1, 512], F32, tag="vmsq")
            nc.vector.tensor_scalar(msq, st_ps[32:33, :], 1.0 / 128, None, ALU.mult)
            nc.vector.tensor_sub(var, msq, var)
            sd = st_pool.tile([1, 512], F32, tag="vsd")
            nc.scalar.activation(sd, var, AF.Sqrt, bias=epsc[0:1, 0:1])
            rr = st_pool.tile([1, 512], BF16, tag="vr")
            with nc.allow_low_precision(reason="bf16 LN rstd; tol 2e-2"):
                nc.vector.reciprocal(rr, sd)
            rb = ps_lg.tile([128, 512], F32, tag="lg")
            nc.tensor.matmul(rb, ones1x128b, rr, start=True, stop=True)
            mb = ps_lg.tile([128, 512], F32, tag="lg")
            nc.tensor.matmul(mb, ones1x128b, mu, start=True, stop=True)
            tmp = st_pool.tile([128, 512], F32, tag="vtmp")
            nc.vector.tensor_sub(tmp, vraw[:, sl], mb)
            nc.vector.tensor_mul(vhat[:, sl], tmp, rb)
        for ch in range(8):
            sl = slice(ch * 512, (ch + 1) * 512)
            vp = ps_lg.tile([128, 512], F32, tag="lg")
            nc.tensor.matmul(vp, vwf_sb, vhat[:, sl], start=True, stop=True)
            nc.vector.tensor_scalar_add(V_sb[:, sl], vp, vbp_sb[:, 0:1])
        # replicate each patch-cell V value 4x along the row for fast srow mult
        for j in range(8):
            src = V_sb[:, j * 512:(j + 1) * 512] \
                .rearrange("p (h c) -> p h c ()", c=64).broadcast_to([128, 8, 64, 4])
            dst = V_rep[:, j * 2048:(j + 1) * 2048] \
                .rearrange("p (h c f) -> p h c f", c=64, f=4)
            nc.scalar.copy(dst, src)

        # ---------------- main loop ----------------
        qqs = {}     # quad index k -> [128,1024] tile (rows 4k..4k+3; q | q^2)
        srows = {}   # pair index -> [128,512] bf16 tile (rows 2i, 2i+1)
        s32s = {}    # batch -> [48,512] psum stats (mu rows 0:16, msq 32:48)
        r16s = {}
        rm16s = {}

        def phase1(i):
            b, j = i // 16, i % 16
            if i % 2 == 0:
                k = i // 2
                qq = qq_pool.tile([128, 1024], F32R, tag="qq")
                nc.sync.dma_start(out=qq[0:64, :], in_=q_d[:, k * 1024:(k + 1) * 1024])
                nc.scalar.activation(qq[64:128, :], qq[0:64, :], AF.Square)
                qqs[k] = qq
            if j == 0:
                s32s[b] = ps_s.tile([48, 512], F32, tag="s32", name="s32")
            qq = qqs[i // 2]
            sl = slice((i % 2) * 512, (i % 2) * 512 + 512)
            nc.tensor.matmul(s32s[b], stx_sb[:, 48 * j:48 * j + 48],
                             qq[:, sl], start=(j == 0), stop=(j == 15))

        def rowmath(b):
            s32 = s32s.pop(b)
            m2 = st_pool.tile([16, 512], F32, tag="m2")
            nc.scalar.activation(m2, s32[0:16, :], AF.Square)
            varp = st_pool.tile([16, 512], F32, tag="varp")
            nc.vector.tensor_sub(varp, s32[32:48, :], m2)
            sd = st_pool.tile([16, 512], F32, tag="sd")
            nc.scalar.activation(sd, varp, AF.Sqrt, bias=epsc[0:16, 0:1])
            r16 = st_pool.tile([16, 512], BF16, tag="r16")
            with nc.allow_low_precision(reason="bf16 r feeds sigmoid logits; tol 2e-2"):
                nc.vector.reciprocal(r16, sd)
            rm16 = st_pool.tile([16, 512], BF16, tag="rm16")
            nc.vector.tensor_mul(rm16, s32[0:16, :], r16)
            r16s[b], rm16s[b] = r16, rm16

        def phase2(i):
            b, j = i // 16, i % 16
            qq = qqs[i // 2]
            sl = slice((i % 2) * 512, (i % 2) * 512 + 512)
            R = ps_r.tile([128, 512], F32, tag="R")
            nc.tensor.matmul(R, onesel_sb[:, 128 * j:128 * j + 128], r16s[b],
                             start=True, stop=True)
            qtil = x_pool.tile([64, 512], BF16, tag="qtil")
            nc.vector.tensor_mul(qtil, qq[0:64, sl].bitcast(F32), R[0:64, :])
            lg = ps_lg.tile([128, 512], F32, tag="lg")
            nc.tensor.matmul(lg, A_sb, qtil, start=True, stop=False)
            nc.tensor.matmul(lg, negu_sb[:, 128 * j:128 * j + 128], rm16s[b],
                             start=False, stop=True)
            sig = sig_pool.tile([128, 512], BF16, tag="sig")
            nc.scalar.activation(sig, lg, AF.Sigmoid, bias=cb_sb[:, 0:1])
            hy = i // 2
            vr = V_rep[:, hy * 256:(hy + 1) * 256]
            srow = srow_pool.tile([128, 512], BF16, tag="srow")
            for rr_ in range(2):
                nc.vector.tensor_mul(srow[:, rr_ * 256:(rr_ + 1) * 256],
                                     sig[:, rr_ * 256:(rr_ + 1) * 256], vr)
            srows[i] = srow

        def row_slice(r, dx):
            lo, n = (0, 255) if dx == 0 else ((1, 255) if dx == 2 else (0, 256))
            if 0 <= r <= 255:
                t = srows[r // 2]
                base = (r % 2) * 256
                return t[:, base + lo:base + lo + n]
            return zrow[:, lo:lo + n]

        def conv_block(y0):
            k = y0 // 4
            cv = ps_cv.tile([128, 512], F32, tag="cv")
            nc.tensor.matmul(cv, zcol, zline, start=True, stop=False)
            for bi, dx in enumerate((1, 0, 2)):
                for ti, t in enumerate((-1, 0, 1, 2)):
                    blk = bi * 4 + ti
                    wt = cwt_sb[:, blk * 128:(blk + 1) * 128]
                    for p in range(2):
                        rt = row_slice(y0 + 2 * p + t, dx)
                        base = p * 256
                        if dx == 1:
                            nc.tensor.matmul(cv[:, base:base + 256], wt, rt,
                                             start=False, stop=False)
                        elif dx == 0:
                            nc.tensor.matmul(cv[:, base + 1:base + 256], wt, rt,
                                             start=False, stop=False)
                        else:
                            nc.tensor.matmul(cv[:, base:base + 255], wt, rt,
                                             start=False, stop=False)
            qq = qqs.pop(k)
            for p in range(2):
                for s in range(2):
                    last = (p == 1 and s == 1)
                    nc.tensor.matmul(cv[:, p * 256:(p + 1) * 256],
                                     ip_sb[:, s * 128:(s + 1) * 128],
                                     qq[0:64, (2 * p + s) * 256:(2 * p + s + 1) * 256],
                                     start=False, stop=last)
            ot = out_pool.tile([128, 512], BF16, tag="ot")
            nc.vector.tensor_scalar(ot, cv, cbb_sb[:, 0:1], None, ALU.add)
            orows = out_d.rearrange("c (r x) -> c r x", x=256)
            for s in range(2):
                nc.sync.dma_start(
                    out=orows[:, y0 + s:y0 + s + 3:2, :],
                    in_=ot[64 * s:64 * s + 64, :].rearrange("c (p x) -> c p x", x=256))
            for r in list(srows):
                if 2 * r + 1 < y0 + 3:
                    del srows[r]

        for step in range(146):
            if step < 128:
                phase1(step)
            if step >= 15 and (step + 1) % 16 == 0 and step <= 127:
                rowmath((step - 15) // 16)
            p2 = step - 16
            if 0 <= p2 < 128:
                phase2(p2)
            if step >= 18 and step % 2 == 0 and (step - 18) // 2 <= 63:
                conv_block(4 * ((step - 18) // 2))

    nc.finalize()
    return nc


def _fold_weights(qW, qb, vW, vb, K, qn_g, qn_b, vn_g, vn_b, cW, cb):
    f = np.float32
    qW, qb, vW, vb, K = f(qW), f(qb), f(vW), f(vb), f(K)
    qn_g, qn_b, vn_g, vn_b, cW, cb = f(qn_g), f(qn_b), f(vn_g), f(vn_b), f(cW), f(cb)
    scale = np.float32(64.0 ** -0.5)
    qWf = qn_g[:, None] * qW.T                      # [c, co]
    bprime = qb + qW @ qn_b                         # [64]
    A = scale * (qWf @ K.T)                         # [64, 128]
    c_b = scale * (K @ bprime)                      # [128]
    u = A.sum(axis=0)                               # [128]
    stx = np.zeros((128, 768), np.float32)
    for i in range(16):
        stx[0:64, 48 * i + i] = 1.0 / 64
        stx[64:128, 48 * i + 32 + i] = 1.0 / 64
    ipair = np.zeros((64, 256), np.float32)
    ipair[:, 0:64] = np.eye(64, dtype=np.float32)
    ipair[:, 192:256] = np.eye(64, dtype=np.float32)
    vWf = vn_g[:, None] * vW.T                      # [128, 128]
    vbp = vb + vW @ vn_b                            # [128]
    cwt = np.zeros((128, 12, 128), np.float32)
    for bi, dx in enumerate((1, 0, 2)):
        for ti, t in enumerate((-1, 0, 1, 2)):
            blk = bi * 4 + ti
            if 0 <= t + 1 <= 2:
                cwt[:, blk, 0:64] = cW[:, :, t + 1, dx].T
            if 0 <= t <= 2:
                cwt[:, blk, 64:128] = cW[:, :, t, dx].T
    negu16 = np.zeros((16, 2048), np.float32)
    onesel = np.zeros((16, 2048), np.float32)
    for j in range(16):
        negu16[j, 128 * j:128 * j + 128] = -u
        onesel[j, 128 * j:128 * j + 128] = 1.0
    return {
        "Amat": np.ascontiguousarray(A.astype(NPBF16)),
        "stx": np.ascontiguousarray(stx),
        "negu": np.ascontiguousarray(negu16.astype(NPBF16)),
        "onesel": np.ascontiguousarray(onesel.astype(NPBF16)),
        "ipair": np.ascontiguousarray(ipair),
        "cbias": np.ascontiguousarray(c_b.reshape(128, 1)),
        "vwf": np.ascontiguousarray(vWf.astype(NPBF16)),
        "vbp": np.ascontiguousarray(vbp.reshape(128, 1)),
        "cwt": np.ascontiguousarray(cwt.reshape(128, 1536).astype(NPBF16)),
        "cbb": np.ascontiguousarray(np.concatenate([cb, cb]).reshape(128, 1)),
    }


def _run(in_maps, trace=False, **kw):
    if "nc" not in _CACHE:
        _CACHE["nc"] = _build_nc()
    return run_bass_kernel_spmd(_CACHE["nc"], in_maps, list(range(8)),
                                trace=trace, **kw)


def kernel(q, v, qW, qb, vW, vb, K, qn_g, qn_b, vn_g, vn_b, cW, cb):
    base = _fold_weights(qW, qb, vW, vb, K, qn_g, qn_b, vn_g, vn_b, cW, cb)
    in_maps = []
    for i in range(8):
        m = dict(base)
        m["q"] = np.ascontiguousarray(np.float32(q[i]).reshape(64, 65536))
        m["v"] = np.ascontiguousarray(np.float32(v[i]).reshape(128, 4096))
        in_maps.append(m)
    res = _run(in_maps)
    outs = [np.asarray(r["out"]).astype(np.float32).reshape(64, 256, 256)
            for r in res.results]
    return np.stack(outs)


# revision 18
# speedup vs baseline: 2.8287x; 1.2781x over previous
"""LocalPatchAttention Trainium2 kernel.

Data-parallel over batch B=8 across 8 NeuronCores (one image per core).
Per-core pipeline for B,Cq,H,W = 8,64,256,256 / Cv,h,w = 128,64,64.

Transpose-free LayerNorm-attention formulation:
  logits[v,px] = r_px * (A.T q[:,px] - mu_px * u) + cb
with A = scale*(g*qW.T)@K.T prefolded on host, u = colsum(A),
mu/E[q^2] per pixel computed by PE ones-matmuls on float32r views of the
raw f32 q rows (1 cycle/row, no bf16 copy), and the per-pixel row math
(var, 1/sqrt) batched over 16 row-pairs so its DVE/Act cost amortizes.
r is broadcast across the 128 v-channels by a rank-1 ones-matmul; the
single DVE multiply X = lg * R feeds Sigmoid (bias = folded cb).
x_attn = sig * V uses a pre-replicated bf16 V (V_rep) so the multiply
runs in the DVE fast mode. 3x3 conv = 12 PSUM-accumulated bf16 matmuls
per 4 output rows (2-row-deep output packing), conv bias as a K=1
matmul, and the residual q added by two identity matmuls per row pair
(float32r). conv PSUM is copied once to bf16 SBUF and DMAed out as
bf16 (upcast on host).

Activation usage stays inside {Square, Copy, Sigmoid} plus one Sqrt per
16-pair batch, so act-table reloads drop from ~2/pair to 2/batch.
"""

import numpy as np
import ml_dtypes

import concourse.bass as bass
import concourse.bacc as bacc
import concourse.tile as tile
from concourse import mybir
from concourse.bass_utils import run_bass_kernel_spmd

F32 = mybir.dt.float32
F32R = mybir.dt.float32r
F8E4 = mybir.dt.float8e4
BF16 = mybir.dt.bfloat16
AF = mybir.ActivationFunctionType
ALU = mybir.AluOpType
EPS = 1e-5
NPBF16 = ml_dtypes.bfloat16
NPF8 = ml_dtypes.float8_e4m3

_CACHE = {}


def _build_nc():
    nc = bacc.Bacc()
    q_d = nc.declare_dram_parameter("q", [64, 65536], F32R, isOutput=False)
    v_d = nc.declare_dram_parameter("v", [128, 4096], F32, isOutput=False)
    A_d = nc.declare_dram_parameter("Amat", [64, 128], BF16, isOutput=False)
    stx_d = nc.declare_dram_parameter("stx", [128, 768], F32R, isOutput=False)
    negu_d = nc.declare_dram_parameter("negu", [16, 2048], BF16, isOutput=False)
    onesel_d = nc.declare_dram_parameter("onesel", [16, 2048], BF16, isOutput=False)
    ip_d = nc.declare_dram_parameter("ipair", [64, 64], F32R, isOutput=False)
    cb_d = nc.declare_dram_parameter("cbias", [128, 1], F32, isOutput=False)
    vwf_d = nc.declare_dram_parameter("vwf", [128, 128], BF16, isOutput=False)
    vbp_d = nc.declare_dram_parameter("vbp", [128, 1], F32, isOutput=False)
    cwt_d = nc.declare_dram_parameter("cwt", [128, 1536], F8E4, isOutput=False)
    cbb_d = nc.declare_dram_parameter("cbb", [64, 1], F32, isOutput=False)
    out_d = nc.declare_dram_parameter("out", [64, 65536], BF16, isOutput=True)

    with tile.TileContext(nc) as tc, \
         tc.tile_pool(name="const", bufs=1) as cpool, \
         tc.tile_pool(name="vwork", bufs=1) as vpool, \
         tc.tile_pool(name="qq", bufs=11) as qq_pool, \
         tc.tile_pool(name="stat", bufs=2) as st_pool, \
         tc.tile_pool(name="xt", bufs=3) as x_pool, \
         tc.tile_pool(name="sig", bufs=3) as sig_pool, \
         tc.tile_pool(name="srow", bufs=12) as srow_pool, \
         tc.tile_pool(name="outp", bufs=3) as out_pool, \
         tc.tile_pool(name="ps_s", bufs=2, space="PSUM") as ps_s, \
         tc.tile_pool(name="ps_lg", bufs=2, space="PSUM") as ps_lg, \
         tc.tile_pool(name="ps_r", bufs=2, space="PSUM") as ps_r, \
         tc.tile_pool(name="ps_cv", bufs=2, space="PSUM") as ps_cv:

        def const_tile(shape, dtype, tag, src):
            t = cpool.tile(shape, dtype, tag=tag)
            nc.sync.dma_start(out=t, in_=src[:, :])
            return t

        A_sb = const_tile([64, 128], BF16, "A", A_d)
        stx_sb = const_tile([128, 768], F32R, "stx", stx_d)
        negu_sb = const_tile([16, 2048], BF16, "negu", negu_d)
        onesel_sb = const_tile([16, 2048], BF16, "onesel", onesel_d)
        ip_sb = const_tile([64, 64], F32R, "ip", ip_d)
        cb_sb = const_tile([128, 1], F32, "cb", cb_d)
        vwf_sb = const_tile([128, 128], BF16, "vwf", vwf_d)
        vbp_sb = const_tile([128, 1], F32, "vbp", vbp_d)
        cwt_sb = const_tile([128, 1536], F8E4, "cwt", cwt_d)
        cbb_sb = const_tile([64, 1], F32, "cbb", cbb_d)

        ones1x128b = cpool.tile([1, 128], BF16, tag="o1x")
        nc.vector.memset(ones1x128b, 1.0)
        ones128b = cpool.tile([128, 1], BF16, tag="o128")
        nc.vector.memset(ones128b, 1.0)
        epsc = cpool.tile([128, 1], F32, tag="eps")
        nc.vector.memset(epsc, EPS)

        # ---------------- V path (once per core) ----------------
        vraw = vpool.tile([128, 4096], F32, tag="vraw")
        vb16 = vpool.tile([128, 4096], BF16, tag="vb16")
        vsqb = vpool.tile([128, 4096], BF16, tag="vsqb")
        vhat = vpool.tile([128, 4096], BF16, tag="vhat")
        V_sb = vpool.tile([128, 4096], F32, tag="V")
        V_rep = vpool.tile([128, 16384], BF16, tag="Vrep")
        for ch in range(8):
            sl = slice(ch * 512, (ch + 1) * 512)
            nc.sync.dma_start(out=vraw[:, sl], in_=v_d[:, sl])
            nc.scalar.copy(vb16[:, sl], vraw[:, sl])
            nc.scalar.activation(vsqb[:, sl], vb16[:, sl], AF.Square)
            st_ps = ps_r.tile([128, 512], F32, tag="R")
            nc.tensor.matmul(st_ps[0:1, :], ones128b, vb16[:, sl],
                             start=True, stop=True)
            nc.tensor.matmul(st_ps[32:33, :], ones128b, vsqb[:, sl],
                             start=True, stop=True)
            mu = st_pool.tile([1, 512], BF16, tag="vmu")
            with nc.allow_low_precision(reason="bf16 LN mean; tol 2e-2"):
                nc.vector.tensor_scalar_mul(mu, st_ps[0:1, :], 1.0 / 128)
            var = st_pool.tile([1, 512], F32, tag="vvar")
            nc.vector.tensor_mul(var, mu, mu)
            msq = st_pool.tile([1, 512], F32, tag="vmsq")
            nc.vector.tensor_scalar(msq, st_ps[32:33, :], 1.0 / 128, None, ALU.mult)
            nc.vector.tensor_sub(var, msq, var)
            sd = st_pool.tile([1, 512], F32, tag="vsd")
            nc.scalar.activation(sd, var, AF.Sqrt, bias=epsc[0:1, 0:1])
            rr = st_pool.tile([1, 512], BF16, tag="vr")
            with nc.allow_low_precision(reason="bf16 LN rstd; tol 2e-2"):
                nc.vector.reciprocal(rr, sd)
            rb = ps_lg.tile([128, 512], F32, tag="lg")
            nc.tensor.matmul(rb, ones1x128b, rr, start=True, stop=True)
            mb = ps_lg.tile([128, 512], F32, tag="lg")
            nc.tensor.matmul(mb, ones1x128b, mu, start=True, stop=True)
            tmp = st_pool.tile([128, 512], F32, tag="vtmp")
            nc.vector.tensor_sub(tmp, vraw[:, sl], mb)
            nc.vector.tensor_mul(vhat[:, sl], tmp, rb)
        for ch in range(8):
            sl = slice(ch * 512, (ch + 1) * 512)
            vp = ps_lg.tile([128, 512], F32, tag="lg")
            nc.tensor.matmul(vp, vwf_sb, vhat[:, sl], start=True, stop=True)
            nc.vector.tensor_scalar_add(V_sb[:, sl], vp, vbp_sb[:, 0:1])
        # replicate each patch-cell V value 4x along the row for fast srow mult
        for j in range(8):
            src = V_sb[:, j * 512:(j + 1) * 512] \
                .rearrange("p (h c) -> p h c ()", c=64).broadcast_to([128, 8, 64, 4])
            dst = V_rep[:, j * 2048:(j + 1) * 2048] \
                .rearrange("p (h c f) -> p h c f", c=64, f=4)
            nc.scalar.copy(dst, src)

        # ---------------- main loop ----------------
        qqs = {}     # quad index k -> [128,1024] tile (rows 4k..4k+3; q | q^2)
        srows = {}   # pair index -> [128,512] bf16 tile (rows 2i, 2i+1)
        s32s = {}    # batch -> [48,512] psum stats (mu rows 0:16, msq 32:48)
        r16s = {}
        rm16s = {}

        def phase1(i):
            b, j = i // 16, i % 16
            if i % 2 == 0:
                k = i // 2
                qq = qq_pool.tile([128, 1024], F32R, tag="qq")
                nc.sync.dma_start(out=qq[0:64, :], in_=q_d[:, k * 1024:(k + 1) * 1024])
                nc.scalar.activation(qq[64:128, :], qq[0:64, :], AF.Square)
                qqs[k] = qq
            if j == 0:
                s32s[b] = ps_s.tile([48, 512], F32, tag="s32", name="s32")
            qq = qqs[i // 2]
            sl = slice((i % 2) * 512, (i % 2) * 512 + 512)
            nc.tensor.matmul(s32s[b], stx_sb[:, 48 * j:48 * j + 48],
                             qq[:, sl], start=(j == 0), stop=(j == 15))

        def rowmath(b):
            s32 = s32s.pop(b)
            m2 = st_pool.tile([16, 512], F32, tag="m2")
            nc.scalar.activation(m2, s32[0:16, :], AF.Square)
            varp = st_pool.tile([16, 512], F32, tag="varp")
            nc.vector.tensor_sub(varp, s32[32:48, :], m2)
            sd = st_pool.tile([16, 512], F32, tag="sd")
            nc.scalar.activation(sd, varp, AF.Sqrt, bias=epsc[0:16, 0:1])
            r16 = st_pool.tile([16, 512], BF16, tag="r16")
            with nc.allow_low_precision(reason="bf16 r feeds sigmoid logits; tol 2e-2"):
                nc.vector.reciprocal(r16, sd)
            rm16 = st_pool.tile([16, 512], BF16, tag="rm16")
            nc.vector.tensor_mul(rm16, s32[0:16, :], r16)
            r16s[b], rm16s[b] = r16, rm16

        def phase2(i):
            b, j = i // 16, i % 16
            qq = qqs[i // 2]
            sl = slice((i % 2) * 512, (i % 2) * 512 + 512)
            R = ps_r.tile([128, 512], F32, tag="R")
            nc.tensor.matmul(R, onesel_sb[:, 128 * j:128 * j + 128], r16s[b],
                             start=True, stop=True)
            qtil = x_pool.tile([64, 512], BF16, tag="qtil")
            nc.vector.tensor_mul(qtil, qq[0:64, sl].bitcast(F32), R[0:64, :])
            lg = ps_lg.tile([128, 512], F32, tag="lg")
            nc.tensor.matmul(lg, A_sb, qtil, start=True, stop=False)
            nc.tensor.matmul(lg, negu_sb[:, 128 * j:128 * j + 128], rm16s[b],
                             start=False, stop=True)
            sig = sig_pool.tile([128, 512], BF16, tag="sig")
            nc.scalar.activation(sig, lg, AF.Sigmoid, bias=cb_sb[:, 0:1])
            hy = i // 2
            vr = V_rep[:, hy * 256:(hy + 1) * 256]
            srow = srow_pool.tile([128, 512], F8E4, tag="srow")
            with nc.allow_low_precision(reason="fp8 conv input; fp8 conv sim err 7e-3, tol 2e-2"):
                for rr_ in range(2):
                    nc.vector.tensor_mul(srow[:, rr_ * 256:(rr_ + 1) * 256],
                                         sig[:, rr_ * 256:(rr_ + 1) * 256], vr)
            srows[i] = srow

        DR = mybir.MatmulPerfMode.DoubleRow

        def conv_block(i):
            # rows r0=2i (even), r1=2i+1 (odd); unpacked out [64ch, 2x256]
            cv = ps_cv.tile([64, 512], F32, tag="cv")
            sl = slice((i % 2) * 512, (i % 2) * 512 + 512)
            qq = qqs[i // 2]
            nc.tensor.matmul(cv, ip_sb, qq[0:64, sl], start=True, stop=False)
            mms = []
            sp = srows[i].rearrange("p (s x) -> p s x", s=2)
            for dxi in range(3):
                lo, olo, n = ((0, 1, 255), (0, 0, 256), (1, 0, 255))[dxi]
                for rpar in range(2):
                    base = rpar * 256
                    out = cv[:, base + olo:base + olo + n]
                    wp = cwt_sb[:, (dxi * 4 + rpar) * 128:(dxi * 4 + rpar + 1) * 128] \
                        .rearrange("p (s m) -> p s m", s=2)
                    mms.append((out, wp, sp[:, :, lo:lo + n]))
                    if rpar == 0 and i > 0:
                        rs = srows[i - 1][:, 256 + lo:256 + lo + n] \
                            .rearrange("p x -> p () x").broadcast_to([128, 2, n])
                    elif rpar == 1 and i < 127:
                        rs = srows[i + 1][:, lo:lo + n] \
                            .rearrange("p x -> p () x").broadcast_to([128, 2, n])
                    else:
                        continue
                    ws = cwt_sb[:, (dxi * 4 + 2 + rpar) * 128:(dxi * 4 + 3 + rpar) * 128] \
                        .rearrange("p (s m) -> p s m", s=2)
                    mms.append((out, ws, rs))
            for mi, (out, w, rhs) in enumerate(mms):
                nc.tensor.matmul(out, w, rhs, start=False, stop=(mi == len(mms) - 1),
                                 perf_mode=DR)
            if i % 2 == 1:
                qqs.pop(i // 2)
            ot = out_pool.tile([64, 512], BF16, tag="ot")
            if i % 2 == 0:
                nc.vector.tensor_scalar(ot, cv, cbb_sb[:, 0:1], None, ALU.add)
            else:
                nc.scalar.activation(ot, cv, AF.Identity, bias=cbb_sb[:, 0:1])
            nc.sync.dma_start(out=out_d[:, 2 * i * 256:(2 * i + 2) * 256], in_=ot)
            for r in list(srows):
                if r < i - 1:
                    del srows[r]

        for step in range(146):
            if step < 128:
                phase1(step)
            if step >= 15 and (step + 1) % 16 == 0 and step <= 127:
                rowmath((step - 15) // 16)
            p2 = step - 16
            if 0 <= p2 < 128:
                phase2(p2)
            if 17 <= step <= 144:
                conv_block(step - 17)

    nc.finalize()
    return nc


def _fold_weights(qW, qb, vW, vb, K, qn_g, qn_b, vn_g, vn_b, cW, cb):
    f = np.float32
    qW, qb, vW, vb, K = f(qW), f(qb), f(vW), f(vb), f(K)
    qn_g, qn_b, vn_g, vn_b, cW, cb = f(qn_g), f(qn_b), f(vn_g), f(vn_b), f(cW), f(cb)
    scale = np.float32(64.0 ** -0.5)
    qWf = qn_g[:, None] * qW.T                      # [c, co]
    bprime = qb + qW @ qn_b                         # [64]
    A = scale * (qWf @ K.T)                         # [64, 128]
    c_b = scale * (K @ bprime)                      # [128]
    u = A.sum(axis=0)                               # [128]
    stx = np.zeros((128, 768), np.float32)
    for i in range(16):
        stx[0:64, 48 * i + i] = 1.0 / 64
        stx[64:128, 48 * i + 32 + i] = 1.0 / 64
    ipair = np.eye(64, dtype=np.float32)
    vWf = vn_g[:, None] * vW.T                      # [128, 128]
    vbp = vb + vW @ vn_b                            # [128]
    # fp8 DoubleRow conv weights: blk = dxi*4 + kind, each [2, 64] (s, m)
    # kind 0 pair-even (ty1, ty2); 1 pair-odd (ty0, ty1);
    # kind 2 single-even (ty0, 0); 3 single-odd (ty2, 0)
    cwt = np.zeros((128, 12, 2, 64), np.float32)
    for dxi in range(3):
        W = [cW[:, :, ty, dxi].T for ty in range(3)]  # [128, 64] each
        cwt[:, dxi * 4 + 0, 0], cwt[:, dxi * 4 + 0, 1] = W[1], W[2]
        cwt[:, dxi * 4 + 1, 0], cwt[:, dxi * 4 + 1, 1] = W[0], W[1]
        cwt[:, dxi * 4 + 2, 0] = W[0]
        cwt[:, dxi * 4 + 3, 0] = W[2]
    negu16 = np.zeros((16, 2048), np.float32)
    onesel = np.zeros((16, 2048), np.float32)
    for j in range(16):
        negu16[j, 128 * j:128 * j + 128] = -u
        onesel[j, 128 * j:128 * j + 128] = 1.0
    return {
        "Amat": np.ascontiguousarray(A.astype(NPBF16)),
        "stx": np.ascontiguousarray(stx),
        "negu": np.ascontiguousarray(negu16.astype(NPBF16)),
        "onesel": np.ascontiguousarray(onesel.astype(NPBF16)),
        "ipair": np.ascontiguousarray(ipair),
        "cbias": np.ascontiguousarray(c_b.reshape(128, 1)),
        "vwf": np.ascontiguousarray(vWf.astype(NPBF16)),
        "vbp": np.ascontiguousarray(vbp.reshape(128, 1)),
        "cwt": np.ascontiguousarray(cwt.reshape(128, 1536).astype(NPF8)),
        "cbb": np.ascontiguousarray(cb.reshape(64, 1)),
    }


def _run(in_maps, trace=False, **kw):
    if "nc" not in _CACHE:
        _CACHE["nc"] = _build_nc()
    return run_bass_kernel_spmd(_CACHE["nc"], in_maps, list(range(8)),
                                trace=trace, **kw)


def kernel(q, v, qW, qb, vW, vb, K, qn_g, qn_b, vn_g, vn_b, cW, cb):
    base = _fold_weights(qW, qb, vW, vb, K, qn_g, qn_b, vn_g, vn_b, cW, cb)
    in_maps = []
    for i in range(8):
        m = dict(base)
        m["q"] = np.ascontiguousarray(np.float32(q[i]).reshape(64, 65536))
        m["v"] = np.ascontiguousarray(np.float32(v[i]).reshape(128, 4096))
        in_maps.append(m)
    res = _run(in_maps)
    outs = [np.asarray(r["out"]).astype(np.float32).reshape(64, 256, 256)
            for r in res.results]
    return np.stack(outs)


# revision 19
# speedup vs baseline: 2.8791x; 1.0178x over previous
"""LocalPatchAttention Trainium2 kernel.

Data-parallel over batch B=8 across 8 NeuronCores (one image per core).
Per-core pipeline for B,Cq,H,W = 8,64,256,256 / Cv,h,w = 128,64,64.

Transpose-free LayerNorm-attention formulation:
  logits[v,px] = r_px * (A.T q[:,px] - mu_px * u) + cb
with A = scale*(g*qW.T)@K.T prefolded on host, u = colsum(A),
mu/E[q^2] per pixel computed by PE ones-matmuls on float32r views of the
raw f32 q rows (1 cycle/row, no bf16 copy), and the per-pixel row math
(var, 1/sqrt) batched over 16 row-pairs so its DVE/Act cost amortizes.
r is broadcast across the 128 v-channels by a rank-1 ones-matmul; the
single DVE multiply X = lg * R feeds Sigmoid (bias = folded cb).
x_attn = sig * V uses a pre-replicated bf16 V (V_rep) so the multiply
runs in the DVE fast mode. 3x3 conv = 12 PSUM-accumulated bf16 matmuls
per 4 output rows (2-row-deep output packing), conv bias as a K=1
matmul, and the residual q added by two identity matmuls per row pair
(float32r). conv PSUM is copied once to bf16 SBUF and DMAed out as
bf16 (upcast on host).

Activation usage stays inside {Square, Copy, Sigmoid} plus one Sqrt per
16-pair batch, so act-table reloads drop from ~2/pair to 2/batch.
"""

import numpy as np
import ml_dtypes

import concourse.bass as bass
import concourse.bacc as bacc
import concourse.tile as tile
from concourse import mybir
from concourse.bass_utils import run_bass_kernel_spmd

F32 = mybir.dt.float32
F32R = mybir.dt.float32r
F8E4 = mybir.dt.float8e4
BF16 = mybir.dt.bfloat16
AF = mybir.ActivationFunctionType
ALU = mybir.AluOpType
EPS = 1e-5
NPBF16 = ml_dtypes.bfloat16
NPF8 = ml_dtypes.float8_e4m3

_CACHE = {}


def _build_nc():
    nc = bacc.Bacc()
    q_d = nc.declare_dram_parameter("q", [64, 65536], F32R, isOutput=False)
    v_d = nc.declare_dram_parameter("v", [128, 4096], F32, isOutput=False)
    A_d = nc.declare_dram_parameter("Amat", [64, 128], BF16, isOutput=False)
    stx_d = nc.declare_dram_parameter("stx", [128, 768], F32R, isOutput=False)
    negu_d = nc.declare_dram_parameter("negu", [16, 2048], BF16, isOutput=False)
    onesel_d = nc.declare_dram_parameter("onesel", [16, 2048], BF16, isOutput=False)
    ip_d = nc.declare_dram_parameter("ipair", [64, 64], F32R, isOutput=False)
    cb_d = nc.declare_dram_parameter("cbias", [128, 1], F32, isOutput=False)
    vwf_d = nc.declare_dram_parameter("vwf", [128, 128], BF16, isOutput=False)
    vbp_d = nc.declare_dram_parameter("vbp", [128, 1], F32, isOutput=False)
    cwt_d = nc.declare_dram_parameter("cwt", [128, 1536], F8E4, isOutput=False)
    cbb_d = nc.declare_dram_parameter("cbb", [64, 1], F32, isOutput=False)
    out_d = nc.declare_dram_parameter("out", [64, 65536], BF16, isOutput=True)

    with tile.TileContext(nc) as tc, \
         tc.tile_pool(name="const", bufs=1) as cpool, \
         tc.tile_pool(name="vwork", bufs=1) as vpool, \
         tc.tile_pool(name="qq", bufs=11) as qq_pool, \
         tc.tile_pool(name="stat", bufs=2) as st_pool, \
         tc.tile_pool(name="xt", bufs=3) as x_pool, \
         tc.tile_pool(name="sig", bufs=3) as sig_pool, \
         tc.tile_pool(name="srow", bufs=12) as srow_pool, \
         tc.tile_pool(name="outp", bufs=3) as out_pool, \
         tc.tile_pool(name="ps_s", bufs=2, space="PSUM") as ps_s, \
         tc.tile_pool(name="ps_lg", bufs=2, space="PSUM") as ps_lg, \
         tc.tile_pool(name="ps_r", bufs=2, space="PSUM") as ps_r, \
         tc.tile_pool(name="ps_cv", bufs=2, space="PSUM") as ps_cv:

        def const_tile(shape, dtype, tag, src):
            t = cpool.tile(shape, dtype, tag=tag)
            nc.sync.dma_start(out=t, in_=src[:, :])
            return t

        A_sb = const_tile([64, 128], BF16, "A", A_d)
        stx_sb = const_tile([128, 768], F32R, "stx", stx_d)
        negu_sb = const_tile([16, 2048], BF16, "negu", negu_d)
        onesel_sb = const_tile([16, 2048], BF16, "onesel", onesel_d)
        ip_sb = const_tile([64, 64], F32R, "ip", ip_d)
        cb_sb = const_tile([128, 1], F32, "cb", cb_d)
        vwf_sb = const_tile([128, 128], BF16, "vwf", vwf_d)
        vbp_sb = const_tile([128, 1], F32, "vbp", vbp_d)
        cwt_sb = const_tile([128, 1536], F8E4, "cwt", cwt_d)
        cbb_sb = const_tile([64, 1], F32, "cbb", cbb_d)

        ones1x128b = cpool.tile([1, 128], BF16, tag="o1x")
        nc.vector.memset(ones1x128b, 1.0)
        ones128b = cpool.tile([128, 1], BF16, tag="o128")
        nc.vector.memset(ones128b, 1.0)
        epsc = cpool.tile([128, 1], F32, tag="eps")
        nc.vector.memset(epsc, EPS)

        # ---------------- V path (once per core) ----------------
        vraw = vpool.tile([128, 4096], F32, tag="vraw")
        vb16 = vpool.tile([128, 4096], BF16, tag="vb16")
        vsqb = vpool.tile([128, 4096], BF16, tag="vsqb")
        vhat = vpool.tile([128, 4096], BF16, tag="vhat")
        V_sb = vpool.tile([128, 4096], F32, tag="V")
        V_rep = vpool.tile([128, 16384], BF16, tag="Vrep")
        for ch in range(8):
            sl = slice(ch * 512, (ch + 1) * 512)
            nc.sync.dma_start(out=vraw[:, sl], in_=v_d[:, sl])
            nc.scalar.copy(vb16[:, sl], vraw[:, sl])
            nc.scalar.activation(vsqb[:, sl], vb16[:, sl], AF.Square)
            st_ps = ps_r.tile([128, 512], F32, tag="R")
            nc.tensor.matmul(st_ps[0:1, :], ones128b, vb16[:, sl],
                             start=True, stop=True)
            nc.tensor.matmul(st_ps[32:33, :], ones128b, vsqb[:, sl],
                             start=True, stop=True)
            mu = st_pool.tile([1, 512], BF16, tag="vmu")
            with nc.allow_low_precision(reason="bf16 LN mean; tol 2e-2"):
                nc.vector.tensor_scalar_mul(mu, st_ps[0:1, :], 1.0 / 128)
            var = st_pool.tile([1, 512], F32, tag="vvar")
            nc.vector.tensor_mul(var, mu, mu)
            msq = st_pool.tile([1, 512], F32, tag="vmsq")
            nc.vector.tensor_scalar(msq, st_ps[32:33, :], 1.0 / 128, None, ALU.mult)
            nc.vector.tensor_sub(var, msq, var)
            sd = st_pool.tile([1, 512], F32, tag="vsd")
            nc.scalar.activation(sd, var, AF.Sqrt, bias=epsc[0:1, 0:1])
            rr = st_pool.tile([1, 512], BF16, tag="vr")
            with nc.allow_low_precision(reason="bf16 LN rstd; tol 2e-2"):
                nc.vector.reciprocal(rr, sd)
            rb = ps_lg.tile([128, 512], F32, tag="lg")
            nc.tensor.matmul(rb, ones1x128b, rr, start=True, stop=True)
            mb = ps_lg.tile([128, 512], F32, tag="lg")
            nc.tensor.matmul(mb, ones1x128b, mu, start=True, stop=True)
            tmp = st_pool.tile([128, 512], F32, tag="vtmp")
            nc.vector.tensor_sub(tmp, vraw[:, sl], mb)
            nc.vector.tensor_mul(vhat[:, sl], tmp, rb)
        for ch in range(8):
            sl = slice(ch * 512, (ch + 1) * 512)
            vp = ps_lg.tile([128, 512], F32, tag="lg")
            nc.tensor.matmul(vp, vwf_sb, vhat[:, sl], start=True, stop=True)
            nc.vector.tensor_scalar_add(V_sb[:, sl], vp, vbp_sb[:, 0:1])
        # replicate each patch-cell V value 4x along the row for fast srow mult
        for j in range(8):
            src = V_sb[:, j * 512:(j + 1) * 512] \
                .rearrange("p (h c) -> p h c ()", c=64).broadcast_to([128, 8, 64, 4])
            dst = V_rep[:, j * 2048:(j + 1) * 2048] \
                .rearrange("p (h c f) -> p h c f", c=64, f=4)
            nc.scalar.copy(dst, src)

        # ---------------- main loop ----------------
        qqs = {}     # quad index k -> [128,1024] tile (rows 4k..4k+3; q | q^2)
        srows = {}   # pair index -> [128,512] bf16 tile (rows 2i, 2i+1)
        s32s = {}    # batch -> [48,512] psum stats (mu rows 0:16, msq 32:48)
        r16s = {}
        rm16s = {}

        def phase1(i):
            b, j = i // 16, i % 16
            if i % 2 == 0:
                k = i // 2
                qq = qq_pool.tile([128, 1024], F32R, tag="qq")
                nc.sync.dma_start(out=qq[0:64, :], in_=q_d[:, k * 1024:(k + 1) * 1024])
                nc.gpsimd.tensor_mul(qq[64:128, :], qq[0:64, :], qq[0:64, :])
                qqs[k] = qq
            if j == 0:
                s32s[b] = ps_s.tile([48, 512], F32, tag="s32", name="s32")
            qq = qqs[i // 2]
            sl = slice((i % 2) * 512, (i % 2) * 512 + 512)
            nc.tensor.matmul(s32s[b], stx_sb[:, 48 * j:48 * j + 48],
                             qq[:, sl], start=(j == 0), stop=(j == 15))

        def rowmath(b):
            s32 = s32s.pop(b)
            m2 = st_pool.tile([16, 512], F32, tag="m2")
            nc.scalar.activation(m2, s32[0:16, :], AF.Square)
            varp = st_pool.tile([16, 512], F32, tag="varp")
            nc.vector.tensor_sub(varp, s32[32:48, :], m2)
            sd = st_pool.tile([16, 512], F32, tag="sd")
            nc.scalar.activation(sd, varp, AF.Sqrt, bias=epsc[0:16, 0:1])
            r16 = st_pool.tile([16, 512], BF16, tag="r16")
            with nc.allow_low_precision(reason="bf16 r feeds sigmoid logits; tol 2e-2"):
                nc.vector.reciprocal(r16, sd)
            rm16 = st_pool.tile([16, 512], BF16, tag="rm16")
            nc.vector.tensor_mul(rm16, s32[0:16, :], r16)
            r16s[b], rm16s[b] = r16, rm16

        def phase2(i):
            b, j = i // 16, i % 16
            qq = qqs[i // 2]
            sl = slice((i % 2) * 512, (i % 2) * 512 + 512)
            R = ps_r.tile([128, 512], F32, tag="R")
            nc.tensor.matmul(R, onesel_sb[:, 128 * j:128 * j + 128], r16s[b],
                             start=True, stop=True)
            qtil = x_pool.tile([64, 512], BF16, tag="qtil")
            nc.vector.tensor_mul(qtil, qq[0:64, sl].bitcast(F32), R[0:64, :])
            lg = ps_lg.tile([128, 512], F32, tag="lg")
            nc.tensor.matmul(lg, A_sb, qtil, start=True, stop=False)
            nc.tensor.matmul(lg, negu_sb[:, 128 * j:128 * j + 128], rm16s[b],
                             start=False, stop=True)
            sig = sig_pool.tile([128, 512], BF16, tag="sig")
            nc.scalar.activation(sig, lg, AF.Sigmoid, bias=cb_sb[:, 0:1])
            hy = i // 2
            vr = V_rep[:, hy * 256:(hy + 1) * 256]
            srow = srow_pool.tile([128, 512], F8E4, tag="srow")
            with nc.allow_low_precision(reason="fp8 conv input; fp8 conv sim err 7e-3, tol 2e-2"):
                nc.vector.tensor_mul(
                    srow.rearrange("p (s x) -> p s x", s=2),
                    sig.rearrange("p (s x) -> p s x", s=2),
                    vr.rearrange("p x -> p () x").broadcast_to([128, 2, 256]))
            srows[i] = srow

        DR = mybir.MatmulPerfMode.DoubleRow

        def conv_block(i):
            # rows r0=2i (even), r1=2i+1 (odd); unpacked out [64ch, 2x256]
            cv = ps_cv.tile([64, 512], F32, tag="cv")
            sl = slice((i % 2) * 512, (i % 2) * 512 + 512)
            qq = qqs[i // 2]
            nc.tensor.matmul(cv, ip_sb, qq[0:64, sl], start=True, stop=False)
            mms = []
            sp = srows[i].rearrange("p (s x) -> p s x", s=2)
            for dxi in range(3):
                lo, olo, n = ((0, 1, 255), (0, 0, 256), (1, 0, 255))[dxi]
                for rpar in range(2):
                    base = rpar * 256
                    out = cv[:, base + olo:base + olo + n]
                    wp = cwt_sb[:, (dxi * 4 + rpar) * 128:(dxi * 4 + rpar + 1) * 128] \
                        .rearrange("p (s m) -> p s m", s=2)
                    mms.append((out, wp, sp[:, :, lo:lo + n]))
                    if rpar == 0 and i > 0:
                        rs = srows[i - 1][:, 256 + lo:256 + lo + n] \
                            .rearrange("p x -> p () x").broadcast_to([128, 2, n])
                    elif rpar == 1 and i < 127:
                        rs = srows[i + 1][:, lo:lo + n] \
                            .rearrange("p x -> p () x").broadcast_to([128, 2, n])
                    else:
                        continue
                    ws = cwt_sb[:, (dxi * 4 + 2 + rpar) * 128:(dxi * 4 + 3 + rpar) * 128] \
                        .rearrange("p (s m) -> p s m", s=2)
                    mms.append((out, ws, rs))
            for mi, (out, w, rhs) in enumerate(mms):
                nc.tensor.matmul(out, w, rhs, start=False, stop=(mi == len(mms) - 1),
                                 perf_mode=DR)
            if i % 2 == 1:
                qqs.pop(i // 2)
            ot = out_pool.tile([64, 512], BF16, tag="ot")
            nc.scalar.activation(ot, cv, AF.Identity, bias=cbb_sb[:, 0:1])
            nc.sync.dma_start(out=out_d[:, 2 * i * 256:(2 * i + 2) * 256], in_=ot)
            for r in list(srows):
                if r < i - 1:
                    del srows[r]

        for step in range(146):
            if step < 128:
                phase1(step)
            if step >= 15 and (step + 1) % 16 == 0 and step <= 127:
                rowmath((step - 15) // 16)
            p2 = step - 16
            if 0 <= p2 < 128:
                phase2(p2)
            if 17 <= step <= 144:
                conv_block(step - 17)

    nc.finalize()
    return nc


def _fold_weights(qW, qb, vW, vb, K, qn_g, qn_b, vn_g, vn_b, cW, cb):
    f = np.float32
    qW, qb, vW, vb, K = f(qW), f(qb), f(vW), f(vb), f(K)
    qn_g, qn_b, vn_g, vn_b, cW, cb = f(qn_g), f(qn_b), f(vn_g), f(vn_b), f(cW), f(cb)
    scale = np.float32(64.0 ** -0.5)
    qWf = qn_g[:, None] * qW.T                      # [c, co]
    bprime = qb + qW @ qn_b                         # [64]
    A = scale * (qWf @ K.T)                         # [64, 128]
    c_b = scale * (K @ bprime)                      # [128]
    u = A.sum(axis=0)                               # [128]
    stx = np.zeros((128, 768), np.float32)
    for i in range(16):
        stx[0:64, 48 * i + i] = 1.0 / 64
        stx[64:128, 48 * i + 32 + i] = 1.0 / 64
    ipair = np.eye(64, dtype=np.float32)
    vWf = vn_g[:, None] * vW.T                      # [128, 128]
    vbp = vb + vW @ vn_b                            # [128]
    # fp8 DoubleRow conv weights: blk = dxi*4 + kind, each [2, 64] (s, m)
    # kind 0 pair-even (ty1, ty2); 1 pair-odd (ty0, ty1);
    # kind 2 single-even (ty0, 0); 3 single-odd (ty2, 0)
    cwt = np.zeros((128, 12, 2, 64), np.float32)
    for dxi in range(3):
        W = [cW[:, :, ty, dxi].T for ty in range(3)]  # [128, 64] each
        cwt[:, dxi * 4 + 0, 0], cwt[:, dxi * 4 + 0, 1] = W[1], W[2]
        cwt[:, dxi * 4 + 1, 0], cwt[:, dxi * 4 + 1, 1] = W[0], W[1]
        cwt[:, dxi * 4 + 2, 0] = W[0]
        cwt[:, dxi * 4 + 3, 0] = W[2]
    negu16 = np.zeros((16, 2048), np.float32)
    onesel = np.zeros((16, 2048), np.float32)
    for j in range(16):
        negu16[j, 128 * j:128 * j + 128] = -u
        onesel[j, 128 * j:128 * j + 128] = 1.0
    return {
        "Amat": np.ascontiguousarray(A.astype(NPBF16)),
        "stx": np.ascontiguousarray(stx),
        "negu": np.ascontiguousarray(negu16.astype(NPBF16)),
        "onesel": np.ascontiguousarray(onesel.astype(NPBF16)),
        "ipair": np.ascontiguousarray(ipair),
        "cbias": np.ascontiguousarray(c_b.reshape(128, 1)),
        "vwf": np.ascontiguousarray(vWf.astype(NPBF16)),
        "vbp": np.ascontiguousarray(vbp.reshape(128, 1)),
        "cwt": np.ascontiguousarray(cwt.reshape(128, 1536).astype(NPF8)),
        "cbb": np.ascontiguousarray(cb.reshape(64, 1)),
    }


def _run(in_maps, trace=False, **kw):
    if "nc" not in _CACHE:
        _CACHE["nc"] = _build_nc()
    return run_bass_kernel_spmd(_CACHE["nc"], in_maps, list(range(8)),
                                trace=trace, **kw)


def kernel(q, v, qW, qb, vW, vb, K, qn_g, qn_b, vn_g, vn_b, cW, cb):
    base = _fold_weights(qW, qb, vW, vb, K, qn_g, qn_b, vn_g, vn_b, cW, cb)
    in_maps = []
    for i in range(8):
        m = dict(base)
        m["q"] = np.ascontiguousarray(np.float32(q[i]).reshape(64, 65536))
        m["v"] = np.ascontiguousarray(np.float32(v[i]).reshape(128, 4096))
        in_maps.append(m)
    res = _run(in_maps)
    outs = [np.asarray(r["out"]).astype(np.float32).reshape(64, 256, 256)
            for r in res.results]
    return np.stack(outs)


# revision 23
# speedup vs baseline: 3.0045x; 1.0436x over previous
"""LocalPatchAttention Trainium2 kernel.

Data-parallel over batch B=8 across 8 NeuronCores (one image per core).
Per-core pipeline for B,Cq,H,W = 8,64,256,256 / Cv,h,w = 128,64,64.

Transpose-free LayerNorm-attention formulation:
  logits[v,px] = r_px * (A.T q[:,px] - mu_px * u) + cb
with A = scale*(g*qW.T)@K.T prefolded on host, u = colsum(A),
mu/E[q^2] per pixel computed by PE ones-matmuls on float32r views of the
raw f32 q rows (1 cycle/row, no bf16 copy), and the per-pixel row math
(var, 1/sqrt) batched over 16 row-pairs so its DVE/Act cost amortizes.
r is broadcast across the 128 v-channels by a rank-1 ones-matmul; the
single DVE multiply X = lg * R feeds Sigmoid (bias = folded cb).
x_attn = sig * V uses a pre-replicated bf16 V (V_rep) so the multiply
runs in the DVE fast mode. 3x3 conv = 12 PSUM-accumulated bf16 matmuls
per 4 output rows (2-row-deep output packing), conv bias as a K=1
matmul, and the residual q added by two identity matmuls per row pair
(float32r). conv PSUM is copied once to bf16 SBUF and DMAed out as
bf16 (upcast on host).

Activation usage stays inside {Square, Copy, Sigmoid} plus one Sqrt per
16-pair batch, so act-table reloads drop from ~2/pair to 2/batch.
"""

import numpy as np
import ml_dtypes

import concourse.bass as bass
import concourse.bacc as bacc
import concourse.tile as tile
from concourse import mybir
from concourse.bass_utils import run_bass_kernel_spmd

F32 = mybir.dt.float32
F32R = mybir.dt.float32r
F8E4 = mybir.dt.float8e4
BF16 = mybir.dt.bfloat16
AF = mybir.ActivationFunctionType
ALU = mybir.AluOpType
EPS = 1e-5
NPBF16 = ml_dtypes.bfloat16
NPF8 = ml_dtypes.float8_e4m3

_CACHE = {}


def _build_nc():
    nc = bacc.Bacc()
    q_d = nc.declare_dram_parameter("q", [64, 65536], F32R, isOutput=False)
    v_d = nc.declare_dram_parameter("v", [128, 4096], F32R, isOutput=False)
    A_d = nc.declare_dram_parameter("Amat", [64, 128], BF16, isOutput=False)
    stx_d = nc.declare_dram_parameter("stx", [128, 768], F32R, isOutput=False)
    negu_d = nc.declare_dram_parameter("negu", [16, 2048], BF16, isOutput=False)
    onesel_d = nc.declare_dram_parameter("onesel", [16, 2048], BF16, isOutput=False)
    ip_d = nc.declare_dram_parameter("ipair", [64, 64], F32R, isOutput=False)
    cb_d = nc.declare_dram_parameter("cbias", [128, 1], F32, isOutput=False)
    vwf_d = nc.declare_dram_parameter("vwf", [128, 128], BF16, isOutput=False)
    vbp_d = nc.declare_dram_parameter("vbp", [128, 1], F32, isOutput=False)
    cwt_d = nc.declare_dram_parameter("cwt", [128, 1536], F8E4, isOutput=False)
    cbb_d = nc.declare_dram_parameter("cbb", [64, 1], F32, isOutput=False)
    onesr_d = nc.declare_dram_parameter("onesr", [128, 1], F32R, isOutput=False)
    out_d = nc.declare_dram_parameter("out", [64, 65536], BF16, isOutput=True)

    with tile.TileContext(nc) as tc, \
         tc.tile_pool(name="const", bufs=1) as cpool, \
         tc.tile_pool(name="vwork", bufs=1) as vpool, \
         tc.tile_pool(name="qq", bufs=11) as qq_pool, \
         tc.tile_pool(name="stat", bufs=2) as st_pool, \
         tc.tile_pool(name="xt", bufs=3) as x_pool, \
         tc.tile_pool(name="sig", bufs=3) as sig_pool, \
         tc.tile_pool(name="srow", bufs=12) as srow_pool, \
         tc.tile_pool(name="outp", bufs=3) as out_pool, \
         tc.tile_pool(name="ps_s", bufs=2, space="PSUM") as ps_s, \
         tc.tile_pool(name="ps_lg", bufs=2, space="PSUM") as ps_lg, \
         tc.tile_pool(name="ps_r", bufs=2, space="PSUM") as ps_r, \
         tc.tile_pool(name="ps_cv", bufs=2, space="PSUM") as ps_cv:

        def const_tile(shape, dtype, tag, src):
            t = cpool.tile(shape, dtype, tag=tag)
            nc.sync.dma_start(out=t, in_=src[:, :])
            return t

        A_sb = const_tile([64, 128], BF16, "A", A_d)
        stx_sb = const_tile([128, 768], F32R, "stx", stx_d)
        negu_sb = const_tile([16, 2048], BF16, "negu", negu_d)
        onesel_sb = const_tile([16, 2048], BF16, "onesel", onesel_d)
        ip_sb = const_tile([64, 64], F32R, "ip", ip_d)
        cb_sb = const_tile([128, 1], F32, "cb", cb_d)
        vwf_sb = const_tile([128, 128], BF16, "vwf", vwf_d)
        vbp_sb = const_tile([128, 1], F32, "vbp", vbp_d)
        cwt_sb = const_tile([128, 1536], F8E4, "cwt", cwt_d)
        cbb_sb = const_tile([64, 1], F32, "cbb", cbb_d)

        ones1x128b = cpool.tile([1, 128], BF16, tag="o1x")
        nc.vector.memset(ones1x128b, 1.0)
        ones128b = cpool.tile([128, 1], BF16, tag="o128")
        nc.vector.memset(ones128b, 1.0)
        ones128r = const_tile([128, 1], F32R, "o128r", onesr_d)
        epsc = cpool.tile([128, 1], F32, tag="eps")
        nc.vector.memset(epsc, EPS)

        # ---------------- V path (once per core) ----------------
        vraw = vpool.tile([128, 4096], F32R, tag="vraw")
        vsqr = vpool.tile([128, 4096], F32R, tag="vsqr")
        vhat = vpool.tile([128, 4096], BF16, tag="vhat")
        V_sb = vpool.tile([128, 4096], F32, tag="V")
        V_rep = vpool.tile([128, 16384], BF16, tag="Vrep")
        for ch in range(8):
            sl = slice(ch * 512, (ch + 1) * 512)
            nc.sync.dma_start(out=vraw[:, sl], in_=v_d[:, sl])
            nc.gpsimd.tensor_mul(vsqr[:, sl], vraw[:, sl], vraw[:, sl])
            st_ps = ps_r.tile([128, 512], F32, tag="R")
            nc.tensor.matmul(st_ps[0:1, :], ones128r, vraw[:, sl],
                             start=True, stop=True)
            st_ps2 = ps_r.tile([128, 512], F32, tag="R")
            nc.tensor.matmul(st_ps2[0:1, :], ones128r, vsqr[:, sl],
                             start=True, stop=True)
            mu = st_pool.tile([1, 512], BF16, tag="vmu")
            with nc.allow_low_precision(reason="bf16 LN mean; tol 2e-2"):
                nc.vector.tensor_scalar_mul(mu, st_ps[0:1, :], 1.0 / 128)
            var = st_pool.tile([1, 512], F32, tag="vvar")
            nc.vector.tensor_mul(var, mu, mu)
            msq = st_pool.tile([1, 512], F32, tag="vmsq")
            nc.vector.tensor_scalar(msq, st_ps2[0:1, :], 1.0 / 128, None, ALU.mult)
            nc.vector.tensor_sub(var, msq, var)
            sd = st_pool.tile([1, 512], F32, tag="vsd")
            nc.scalar.activation(sd, var, AF.Sqrt, bias=epsc[0:1, 0:1])
            rr = st_pool.tile([1, 512], BF16, tag="vr")
            with nc.allow_low_precision(reason="bf16 LN rstd; tol 2e-2"):
                nc.vector.reciprocal(rr, sd)
            rb = ps_lg.tile([128, 512], F32, tag="lg")
            nc.tensor.matmul(rb, ones1x128b, rr, start=True, stop=True)
            mb = ps_lg.tile([128, 512], F32, tag="lg")
            nc.tensor.matmul(mb, ones1x128b, mu, start=True, stop=True)
            tmp = st_pool.tile([128, 512], F32, tag="vtmp")
            nc.vector.tensor_sub(tmp, vraw[:, sl].bitcast(F32), mb)
            nc.vector.tensor_mul(vhat[:, sl], tmp, rb)
        for ch in range(8):
            sl = slice(ch * 512, (ch + 1) * 512)
            vp = ps_lg.tile([128, 512], F32, tag="lg")
            nc.tensor.matmul(vp, vwf_sb, vhat[:, sl], start=True, stop=True)
            nc.vector.tensor_scalar_add(V_sb[:, sl], vp, vbp_sb[:, 0:1])
        # replicate each patch-cell V value 4x along the row for fast srow mult
        for j in range(8):
            src = V_sb[:, j * 512:(j + 1) * 512] \
                .rearrange("p (h c) -> p h c ()", c=64).broadcast_to([128, 8, 64, 4])
            dst = V_rep[:, j * 2048:(j + 1) * 2048] \
                .rearrange("p (h c f) -> p h c f", c=64, f=4)
            nc.gpsimd.tensor_copy(out=dst, in_=src)

        # ---------------- main loop ----------------
        qqs = {}     # quad index k -> [128,1024] tile (rows 4k..4k+3; q | q^2)
        srows = {}   # pair index -> [128,512] bf16 tile (rows 2i, 2i+1)
        s32s = {}    # batch -> [48,512] psum stats (mu rows 0:16, msq 32:48)
        r16s = {}
        rm16s = {}

        def phase1(i):
            b, j = i // 16, i % 16
            if i % 2 == 0:
                k = i // 2
                qq = qq_pool.tile([128, 1024], F32R, tag="qq")
                nc.sync.dma_start(out=qq[0:64, :], in_=q_d[:, k * 1024:(k + 1) * 1024])
                nc.gpsimd.tensor_mul(qq[64:128, :], qq[0:64, :], qq[0:64, :])
                qqs[k] = qq
            if j == 0:
                s32s[b] = ps_s.tile([48, 512], F32, tag="s32", name="s32")
            qq = qqs[i // 2]
            sl = slice((i % 2) * 512, (i % 2) * 512 + 512)
            nc.tensor.matmul(s32s[b], stx_sb[:, 48 * j:48 * j + 48],
                             qq[:, sl], start=(j == 0), stop=(j == 15))

        def rowmath(b):
            s32 = s32s.pop(b)
            m2 = st_pool.tile([16, 512], F32, tag="m2")
            nc.scalar.activation(m2, s32[0:16, :], AF.Square)
            varp = st_pool.tile([16, 512], F32, tag="varp")
            nc.vector.tensor_sub(varp, s32[32:48, :], m2)
            sd = st_pool.tile([16, 512], F32, tag="sd")
            nc.scalar.activation(sd, varp, AF.Sqrt, bias=epsc[0:16, 0:1])
            r16 = st_pool.tile([16, 512], BF16, tag="r16")
            with nc.allow_low_precision(reason="bf16 r feeds sigmoid logits; tol 2e-2"):
                nc.vector.reciprocal(r16, sd)
            rm16 = st_pool.tile([16, 512], BF16, tag="rm16")
            nc.vector.tensor_mul(rm16, s32[0:16, :], r16)
            r16s[b], rm16s[b] = r16, rm16

        def phase2(i):
            b, j = i // 16, i % 16
            qq = qqs[i // 2]
            sl = slice((i % 2) * 512, (i % 2) * 512 + 512)
            R = ps_r.tile([128, 512], F32, tag="R")
            nc.tensor.matmul(R, onesel_sb[:, 128 * j:128 * j + 128], r16s[b],
                             start=True, stop=True)
            qtil = x_pool.tile([64, 512], BF16, tag="qtil")
            nc.vector.tensor_mul(qtil, qq[0:64, sl].bitcast(F32), R[0:64, :])
            lg = ps_lg.tile([128, 512], F32, tag="lg")
            nc.tensor.matmul(lg, A_sb, qtil, start=True, stop=False)
            nc.tensor.matmul(lg, negu_sb[:, 128 * j:128 * j + 128], rm16s[b],
                             start=False, stop=True)
            sig = sig_pool.tile([128, 512], BF16, tag="sig")
            nc.scalar.activation(sig, lg, AF.Sigmoid, bias=cb_sb[:, 0:1])
            hy = i // 2
            vr = V_rep[:, hy * 256:(hy + 1) * 256]
            srow = srow_pool.tile([128, 512], F8E4, tag="srow")
            with nc.allow_low_precision(reason="fp8 conv input; fp8 conv sim err 7e-3, tol 2e-2"):
                nc.vector.tensor_mul(
                    srow.rearrange("p (s x) -> p s x", s=2),
                    sig.rearrange("p (s x) -> p s x", s=2),
                    vr.rearrange("p x -> p () x").broadcast_to([128, 2, 256]))
            srows[i] = srow

        DR = mybir.MatmulPerfMode.DoubleRow

        def conv_block(i):
            # rows r0=2i (even), r1=2i+1 (odd); unpacked out [64ch, 2x256]
            cv = ps_cv.tile([64, 512], F32, tag="cv")
            sl = slice((i % 2) * 512, (i % 2) * 512 + 512)
            qq = qqs[i // 2]
            nc.tensor.matmul(cv, ip_sb, qq[0:64, sl], start=True, stop=False)
            mms = []
            sp = srows[i].rearrange("p (s x) -> p s x", s=2)
            for dxi in range(3):
                lo, olo, n = ((0, 1, 255), (0, 0, 256), (1, 0, 255))[dxi]
                for rpar in range(2):
                    base = rpar * 256
                    out = cv[:, base + olo:base + olo + n]
                    wp = cwt_sb[:, (dxi * 4 + rpar) * 128:(dxi * 4 + rpar + 1) * 128] \
                        .rearrange("p (s m) -> p s m", s=2)
                    mms.append((out, wp, sp[:, :, lo:lo + n]))
                    if rpar == 0 and i > 0:
                        rs = srows[i - 1][:, 256 + lo:256 + lo + n] \
                            .rearrange("p x -> p () x").broadcast_to([128, 2, n])
                    elif rpar == 1 and i < 127:
                        rs = srows[i + 1][:, lo:lo + n] \
                            .rearrange("p x -> p () x").broadcast_to([128, 2, n])
                    else:
                        continue
                    ws = cwt_sb[:, (dxi * 4 + 2 + rpar) * 128:(dxi * 4 + 3 + rpar) * 128] \
                        .rearrange("p (s m) -> p s m", s=2)
                    mms.append((out, ws, rs))
            for mi, (out, w, rhs) in enumerate(mms):
                nc.tensor.matmul(out, w, rhs, start=False, stop=(mi == len(mms) - 1),
                                 perf_mode=DR)
            if i % 2 == 1:
                qqs.pop(i // 2)
            ot = out_pool.tile([64, 512], BF16, tag="ot")
            nc.scalar.activation(ot, cv, AF.Identity, bias=cbb_sb[:, 0:1])
            nc.sync.dma_start(out=out_d[:, 2 * i * 256:(2 * i + 2) * 256], in_=ot)
            for r in list(srows):
                if r < i - 1:
                    del srows[r]

        for step in range(146):
            if step < 128:
                phase1(step)
            if step >= 15 and (step + 1) % 16 == 0 and step <= 127:
                rowmath((step - 15) // 16)
            p2 = step - 16
            if 0 <= p2 < 128:
                phase2(p2)
            if 17 <= step <= 144:
                conv_block(step - 17)

    nc.finalize()
    return nc


def _fold_weights(qW, qb, vW, vb, K, qn_g, qn_b, vn_g, vn_b, cW, cb):
    f = np.float32
    qW, qb, vW, vb, K = f(qW), f(qb), f(vW), f(vb), f(K)
    qn_g, qn_b, vn_g, vn_b, cW, cb = f(qn_g), f(qn_b), f(vn_g), f(vn_b), f(cW), f(cb)
    scale = np.float32(64.0 ** -0.5)
    qWf = qn_g[:, None] * qW.T                      # [c, co]
    bprime = qb + qW @ qn_b                         # [64]
    A = scale * (qWf @ K.T)                         # [64, 128]
    c_b = scale * (K @ bprime)                      # [128]
    u = A.sum(axis=0)                               # [128]
    stx = np.zeros((128, 768), np.float32)
    for i in range(16):
        stx[0:64, 48 * i + i] = 1.0 / 64
        stx[64:128, 48 * i + 32 + i] = 1.0 / 64
    ipair = np.eye(64, dtype=np.float32)
    vWf = vn_g[:, None] * vW.T                      # [128, 128]
    vbp = vb + vW @ vn_b                            # [128]
    # fp8 DoubleRow conv weights: blk = dxi*4 + kind, each [2, 64] (s, m)
    # kind 0 pair-even (ty1, ty2); 1 pair-odd (ty0, ty1);
    # kind 2 single-even (ty0, 0); 3 single-odd (ty2, 0)
    cwt = np.zeros((128, 12, 2, 64), np.float32)
    for dxi in range(3):
        W = [cW[:, :, ty, dxi].T for ty in range(3)]  # [128, 64] each
        cwt[:, dxi * 4 + 0, 0], cwt[:, dxi * 4 + 0, 1] = W[1], W[2]
        cwt[:, dxi * 4 + 1, 0], cwt[:, dxi * 4 + 1, 1] = W[0], W[1]
        cwt[:, dxi * 4 + 2, 0] = W[0]
        cwt[:, dxi * 4 + 3, 0] = W[2]
    negu16 = np.zeros((16, 2048), np.float32)
    onesel = np.zeros((16, 2048), np.float32)
    for j in range(16):
        negu16[j, 128 * j:128 * j + 128] = -u
        onesel[j, 128 * j:128 * j + 128] = 1.0
    return {
        "Amat": np.ascontiguousarray(A.astype(NPBF16)),
        "stx": np.ascontiguousarray(stx),
        "negu": np.ascontiguousarray(negu16.astype(NPBF16)),
        "onesel": np.ascontiguousarray(onesel.astype(NPBF16)),
        "ipair": np.ascontiguousarray(ipair),
        "cbias": np.ascontiguousarray(c_b.reshape(128, 1)),
        "vwf": np.ascontiguousarray(vWf.astype(NPBF16)),
        "vbp": np.ascontiguousarray(vbp.reshape(128, 1)),
        "cwt": np.ascontiguousarray(cwt.reshape(128, 1536).astype(NPF8)),
        "cbb": np.ascontiguousarray(cb.reshape(64, 1)),
        "onesr": np.ascontiguousarray(np.ones((128, 1), np.float32)),
    }


def _run(in_maps, trace=False, **kw):
    if "nc" not in _CACHE:
        _CACHE["nc"] = _build_nc()
    return run_bass_kernel_spmd(_CACHE["nc"], in_maps, list(range(8)),
                                trace=trace, **kw)


def kernel(q, v, qW, qb, vW, vb, K, qn_g, qn_b, vn_g, vn_b, cW, cb):
    base = _fold_weights(qW, qb, vW, vb, K, qn_g, qn_b, vn_g, vn_b, cW, cb)
    in_maps = []
    for i in range(8):
        m = dict(base)
        m["q"] = np.ascontiguousarray(np.float32(q[i]).reshape(64, 65536))
        m["v"] = np.ascontiguousarray(np.float32(v[i]).reshape(128, 4096))
        in_maps.append(m)
    res = _run(in_maps)
    outs = [np.asarray(r["out"]).astype(np.float32).reshape(64, 256, 256)
            for r in res.results]
    return np.stack(outs)


# revision 24
# speedup vs baseline: 3.0729x; 1.0227x over previous
"""LocalPatchAttention Trainium2 kernel.

Data-parallel over batch B=8 across 8 NeuronCores (one image per core).
Per-core pipeline for B,Cq,H,W = 8,64,256,256 / Cv,h,w = 128,64,64.

Transpose-free LayerNorm-attention formulation:
  logits[v,px] = r_px * (A.T q[:,px] - mu_px * u) + cb
with A = scale*(g*qW.T)@K.T prefolded on host, u = colsum(A),
mu/E[q^2] per pixel computed by PE ones-matmuls on float32r views of the
raw f32 q rows (1 cycle/row, no bf16 copy), and the per-pixel row math
(var, 1/sqrt) batched over 16 row-pairs so its DVE/Act cost amortizes.
r is broadcast across the 128 v-channels by a rank-1 ones-matmul; the
single DVE multiply X = lg * R feeds Sigmoid (bias = folded cb).
x_attn = sig * V uses a pre-replicated bf16 V (V_rep) so the multiply
runs in the DVE fast mode. 3x3 conv = 12 PSUM-accumulated bf16 matmuls
per 4 output rows (2-row-deep output packing), conv bias as a K=1
matmul, and the residual q added by two identity matmuls per row pair
(float32r). conv PSUM is copied once to bf16 SBUF and DMAed out as
bf16 (upcast on host).

Activation usage stays inside {Square, Copy, Sigmoid} plus one Sqrt per
16-pair batch, so act-table reloads drop from ~2/pair to 2/batch.
"""

import numpy as np
import ml_dtypes

import concourse.bass as bass
import concourse.bacc as bacc
import concourse.tile as tile
from concourse import mybir
from concourse.bass_utils import run_bass_kernel_spmd

F32 = mybir.dt.float32
F32R = mybir.dt.float32r
F8E4 = mybir.dt.float8e4
BF16 = mybir.dt.bfloat16
AF = mybir.ActivationFunctionType
ALU = mybir.AluOpType
EPS = 1e-5
NPBF16 = ml_dtypes.bfloat16
NPF8 = ml_dtypes.float8_e4m3

_CACHE = {}


def _build_nc():
    nc = bacc.Bacc()
    q_d = nc.declare_dram_parameter("q", [64, 65536], F32R, isOutput=False)
    v_d = nc.declare_dram_parameter("v", [128, 4096], F32R, isOutput=False)
    A_d = nc.declare_dram_parameter("Amat", [64, 128], BF16, isOutput=False)
    stx_d = nc.declare_dram_parameter("stx", [128, 768], F32R, isOutput=False)
    negu_d = nc.declare_dram_parameter("negu", [16, 2048], BF16, isOutput=False)
    onesel_d = nc.declare_dram_parameter("onesel", [16, 2048], BF16, isOutput=False)
    ip_d = nc.declare_dram_parameter("ipair", [64, 64], F32R, isOutput=False)
    cb_d = nc.declare_dram_parameter("cbias", [128, 1], F32, isOutput=False)
    vwf_d = nc.declare_dram_parameter("vwf", [128, 128], BF16, isOutput=False)
    vbp_d = nc.declare_dram_parameter("vbp", [128, 1], F32, isOutput=False)
    cwt_d = nc.declare_dram_parameter("cwt", [128, 1536], F8E4, isOutput=False)
    cbb_d = nc.declare_dram_parameter("cbb", [64, 1], F32, isOutput=False)
    vsel_d = nc.declare_dram_parameter("vsel", [128, 640], F32R, isOutput=False)
    vsel2_d = nc.declare_dram_parameter("vsel2", [8, 1024], BF16, isOutput=False)
    negw1_d = nc.declare_dram_parameter("negw1", [8, 1024], BF16, isOutput=False)
    out_d = nc.declare_dram_parameter("out", [64, 65536], BF16, isOutput=True)

    with tile.TileContext(nc) as tc, \
         tc.tile_pool(name="const", bufs=1) as cpool, \
         tc.tile_pool(name="vwork", bufs=1) as vpool, \
         tc.tile_pool(name="qq", bufs=11) as qq_pool, \
         tc.tile_pool(name="stat", bufs=2) as st_pool, \
         tc.tile_pool(name="xt", bufs=3) as x_pool, \
         tc.tile_pool(name="sig", bufs=3) as sig_pool, \
         tc.tile_pool(name="srow", bufs=12) as srow_pool, \
         tc.tile_pool(name="outp", bufs=3) as out_pool, \
         tc.tile_pool(name="ps_s", bufs=2, space="PSUM") as ps_s, \
         tc.tile_pool(name="ps_lg", bufs=2, space="PSUM") as ps_lg, \
         tc.tile_pool(name="ps_r", bufs=2, space="PSUM") as ps_r, \
         tc.tile_pool(name="ps_cv", bufs=2, space="PSUM") as ps_cv:

        def const_tile(shape, dtype, tag, src):
            t = cpool.tile(shape, dtype, tag=tag)
            nc.sync.dma_start(out=t, in_=src[:, :])
            return t

        A_sb = const_tile([64, 128], BF16, "A", A_d)
        stx_sb = const_tile([128, 768], F32R, "stx", stx_d)
        negu_sb = const_tile([16, 2048], BF16, "negu", negu_d)
        onesel_sb = const_tile([16, 2048], BF16, "onesel", onesel_d)
        ip_sb = const_tile([64, 64], F32R, "ip", ip_d)
        cb_sb = const_tile([128, 1], F32, "cb", cb_d)
        vwf_sb = const_tile([128, 128], BF16, "vwf", vwf_d)
        vbp_sb = const_tile([128, 1], F32, "vbp", vbp_d)
        cwt_sb = const_tile([128, 1536], F8E4, "cwt", cwt_d)
        cbb_sb = const_tile([64, 1], F32, "cbb", cbb_d)

        ones1x128b = cpool.tile([1, 128], BF16, tag="o1x")
        nc.vector.memset(ones1x128b, 1.0)
        ones128b = cpool.tile([128, 1], BF16, tag="o128")
        nc.vector.memset(ones128b, 1.0)
        vsel_sb = const_tile([128, 640], F32R, "vsel", vsel_d)
        vsel2_sb = const_tile([8, 1024], BF16, "vsel2", vsel2_d)
        negw1_sb = const_tile([8, 1024], BF16, "negw1", negw1_d)
        epsc = cpool.tile([128, 1], F32, tag="eps")
        nc.vector.memset(epsc, EPS)

        # ---------------- V path (once per core) ----------------
        # batched stats: vst[0:8]=col-sums/128 per chunk, vst[32:40]=sq-sums/128
        vraw = vpool.tile([128, 4096], F32R, tag="vraw")
        vsqr = vpool.tile([128, 4096], F32R, tag="vsqr")
        V_sb = vpool.tile([128, 4096], F32, tag="V")
        V_rep = vpool.tile([128, 16384], BF16, tag="Vrep")
        vst = ps_s.tile([40, 512], F32, tag="s32", name="vst")
        for ch in range(8):
            sl = slice(ch * 512, (ch + 1) * 512)
            nc.sync.dma_start(out=vraw[:, sl], in_=v_d[:, sl])
            nc.gpsimd.tensor_mul(vsqr[:, sl], vraw[:, sl], vraw[:, sl])
            nc.tensor.matmul(vst, vsel_sb[:, 80 * ch:80 * ch + 40], vraw[:, sl],
                             start=(ch == 0), stop=False)
            nc.tensor.matmul(vst, vsel_sb[:, 80 * ch + 40:80 * ch + 80], vsqr[:, sl],
                             start=False, stop=(ch == 7))
        m2v = st_pool.tile([8, 512], F32, tag="m2v")
        nc.scalar.activation(m2v, vst[0:8, :], AF.Square)
        varv = st_pool.tile([8, 512], F32, tag="varv")
        nc.vector.tensor_sub(varv, vst[32:40, :], m2v)
        sdv = st_pool.tile([8, 512], F32, tag="sdv")
        nc.scalar.activation(sdv, varv, AF.Sqrt, bias=epsc[0:8, 0:1])
        rrv = st_pool.tile([8, 512], BF16, tag="rrv")
        with nc.allow_low_precision(reason="bf16 LN rstd; tol 2e-2"):
            nc.vector.reciprocal(rrv, sdv)
        rmv = st_pool.tile([8, 512], BF16, tag="rmv")
        nc.vector.tensor_mul(rmv, vst[0:8, :], rrv)
        for ch in range(8):
            sl = slice(ch * 512, (ch + 1) * 512)
            rb = ps_lg.tile([128, 512], F32, tag="lg")
            nc.tensor.matmul(rb, vsel2_sb[:, 128 * ch:128 * ch + 128], rrv,
                             start=True, stop=True)
            vt = x_pool.tile([128, 512], BF16, tag="vt")
            nc.vector.tensor_mul(vt, vraw[:, sl].bitcast(F32), rb)
            vp = ps_lg.tile([128, 512], F32, tag="lg")
            nc.tensor.matmul(vp, vwf_sb, vt, start=True, stop=False)
            nc.tensor.matmul(vp, negw1_sb[:, 128 * ch:128 * ch + 128], rmv,
                             start=False, stop=True)
            nc.vector.tensor_scalar_add(V_sb[:, sl], vp, vbp_sb[:, 0:1])
        # replicate each patch-cell V value 4x along the row for fast srow mult
        for j in range(8):
            src = V_sb[:, j * 512:(j + 1) * 512] \
                .rearrange("p (h c) -> p h c ()", c=64).broadcast_to([128, 8, 64, 4])
            dst = V_rep[:, j * 2048:(j + 1) * 2048] \
                .rearrange("p (h c f) -> p h c f", c=64, f=4)
            nc.gpsimd.tensor_copy(out=dst, in_=src)

        # ---------------- main loop ----------------
        qqs = {}     # quad index k -> [128,1024] tile (rows 4k..4k+3; q | q^2)
        srows = {}   # pair index -> [128,512] bf16 tile (rows 2i, 2i+1)
        s32s = {}    # batch -> [48,512] psum stats (mu rows 0:16, msq 32:48)
        r16s = {}
        rm16s = {}

        def phase1(i):
            b, j = i // 16, i % 16
            if i % 2 == 0:
                k = i // 2
                qq = qq_pool.tile([128, 1024], F32R, tag="qq")
                nc.sync.dma_start(out=qq[0:64, :], in_=q_d[:, k * 1024:(k + 1) * 1024])
                nc.gpsimd.tensor_mul(qq[64:128, :], qq[0:64, :], qq[0:64, :])
                qqs[k] = qq
            if j == 0:
                s32s[b] = ps_s.tile([48, 512], F32, tag="s32", name="s32")
            qq = qqs[i // 2]
            sl = slice((i % 2) * 512, (i % 2) * 512 + 512)
            nc.tensor.matmul(s32s[b], stx_sb[:, 48 * j:48 * j + 48],
                             qq[:, sl], start=(j == 0), stop=(j == 15))

        def rowmath(b):
            s32 = s32s.pop(b)
            m2 = st_pool.tile([16, 512], F32, tag="m2")
            nc.scalar.activation(m2, s32[0:16, :], AF.Square)
            varp = st_pool.tile([16, 512], F32, tag="varp")
            nc.vector.tensor_sub(varp, s32[32:48, :], m2)
            sd = st_pool.tile([16, 512], F32, tag="sd")
            nc.scalar.activation(sd, varp, AF.Sqrt, bias=epsc[0:16, 0:1])
            r16 = st_pool.tile([16, 512], BF16, tag="r16")
            with nc.allow_low_precision(reason="bf16 r feeds sigmoid logits; tol 2e-2"):
                nc.vector.reciprocal(r16, sd)
            rm16 = st_pool.tile([16, 512], BF16, tag="rm16")
            nc.vector.tensor_mul(rm16, s32[0:16, :], r16)
            r16s[b], rm16s[b] = r16, rm16

        def phase2(i):
            b, j = i // 16, i % 16
            qq = qqs[i // 2]
            sl = slice((i % 2) * 512, (i % 2) * 512 + 512)
            R = ps_r.tile([128, 512], F32, tag="R")
            nc.tensor.matmul(R, onesel_sb[:, 128 * j:128 * j + 128], r16s[b],
                             start=True, stop=True)
            qtil = x_pool.tile([64, 512], BF16, tag="qtil")
            nc.vector.tensor_mul(qtil, qq[0:64, sl].bitcast(F32), R[0:64, :])
            lg = ps_lg.tile([128, 512], F32, tag="lg")
            nc.tensor.matmul(lg, A_sb, qtil, start=True, stop=False)
            nc.tensor.matmul(lg, negu_sb[:, 128 * j:128 * j + 128], rm16s[b],
                             start=False, stop=True)
            sig = sig_pool.tile([128, 512], BF16, tag="sig")
            nc.scalar.activation(sig, lg, AF.Sigmoid, bias=cb_sb[:, 0:1])
            hy = i // 2
            vr = V_rep[:, hy * 256:(hy + 1) * 256]
            srow = srow_pool.tile([128, 512], F8E4, tag="srow")
            with nc.allow_low_precision(reason="fp8 conv input; fp8 conv sim err 7e-3, tol 2e-2"):
                nc.vector.tensor_mul(
                    srow.rearrange("p (s x) -> p s x", s=2),
                    sig.rearrange("p (s x) -> p s x", s=2),
                    vr.rearrange("p x -> p () x").broadcast_to([128, 2, 256]))
            srows[i] = srow

        DR = mybir.MatmulPerfMode.DoubleRow

        def conv_block(i):
            # rows r0=2i (even), r1=2i+1 (odd); unpacked out [64ch, 2x256]
            cv = ps_cv.tile([64, 512], F32, tag="cv")
            sl = slice((i % 2) * 512, (i % 2) * 512 + 512)
            qq = qqs[i // 2]
            nc.tensor.matmul(cv, ip_sb, qq[0:64, sl], start=True, stop=False)
            mms = []
            sp = srows[i].rearrange("p (s x) -> p s x", s=2)
            for dxi in range(3):
                lo, olo, n = ((0, 1, 255), (0, 0, 256), (1, 0, 255))[dxi]
                for rpar in range(2):
                    base = rpar * 256
                    out = cv[:, base + olo:base + olo + n]
                    wp = cwt_sb[:, (dxi * 4 + rpar) * 128:(dxi * 4 + rpar + 1) * 128] \
                        .rearrange("p (s m) -> p s m", s=2)
                    mms.append((out, wp, sp[:, :, lo:lo + n]))
                    if rpar == 0 and i > 0:
                        rs = srows[i - 1][:, 256 + lo:256 + lo + n] \
                            .rearrange("p x -> p () x").broadcast_to([128, 2, n])
                    elif rpar == 1 and i < 127:
                        rs = srows[i + 1][:, lo:lo + n] \
                            .rearrange("p x -> p () x").broadcast_to([128, 2, n])
                    else:
                        continue
                    ws = cwt_sb[:, (dxi * 4 + 2 + rpar) * 128:(dxi * 4 + 3 + rpar) * 128] \
                        .rearrange("p (s m) -> p s m", s=2)
                    mms.append((out, ws, rs))
            for mi, (out, w, rhs) in enumerate(mms):
                nc.tensor.matmul(out, w, rhs, start=False, stop=(mi == len(mms) - 1),
                                 perf_mode=DR)
            if i % 2 == 1:
                qqs.pop(i // 2)
            ot = out_pool.tile([64, 512], BF16, tag="ot")
            nc.scalar.activation(ot, cv, AF.Identity, bias=cbb_sb[:, 0:1])
            nc.sync.dma_start(out=out_d[:, 2 * i * 256:(2 * i + 2) * 256], in_=ot)
            for r in list(srows):
                if r < i - 1:
                    del srows[r]

        for step in range(146):
            if step < 128:
                phase1(step)
            if step >= 15 and (step + 1) % 16 == 0 and step <= 127:
                rowmath((step - 15) // 16)
            p2 = step - 16
            if 0 <= p2 < 128:
                phase2(p2)
            if 17 <= step <= 144:
                conv_block(step - 17)

    nc.finalize()
    return nc


def _fold_weights(qW, qb, vW, vb, K, qn_g, qn_b, vn_g, vn_b, cW, cb):
    f = np.float32
    qW, qb, vW, vb, K = f(qW), f(qb), f(vW), f(vb), f(K)
    qn_g, qn_b, vn_g, vn_b, cW, cb = f(qn_g), f(qn_b), f(vn_g), f(vn_b), f(cW), f(cb)
    scale = np.float32(64.0 ** -0.5)
    qWf = qn_g[:, None] * qW.T                      # [c, co]
    bprime = qb + qW @ qn_b                         # [64]
    A = scale * (qWf @ K.T)                         # [64, 128]
    c_b = scale * (K @ bprime)                      # [128]
    u = A.sum(axis=0)                               # [128]
    stx = np.zeros((128, 768), np.float32)
    for i in range(16):
        stx[0:64, 48 * i + i] = 1.0 / 64
        stx[64:128, 48 * i + 32 + i] = 1.0 / 64
    ipair = np.eye(64, dtype=np.float32)
    vWf = vn_g[:, None] * vW.T                      # [128, 128]
    vbp = vb + vW @ vn_b                            # [128]
    w1 = vWf.sum(axis=0)                            # [128]
    vsel = np.zeros((128, 640), np.float32)
    vsel2 = np.zeros((8, 1024), np.float32)
    negw1 = np.zeros((8, 1024), np.float32)
    for j in range(8):
        vsel[:, 80 * j + j] = 1.0 / 128
        vsel[:, 80 * j + 40 + 32 + j] = 1.0 / 128
        vsel2[j, 128 * j:128 * j + 128] = 1.0
        negw1[j, 128 * j:128 * j + 128] = -w1
    # fp8 DoubleRow conv weights: blk = dxi*4 + kind, each [2, 64] (s, m)
    # kind 0 pair-even (ty1, ty2); 1 pair-odd (ty0, ty1);
    # kind 2 single-even (ty0, 0); 3 single-odd (ty2, 0)
    cwt = np.zeros((128, 12, 2, 64), np.float32)
    for dxi in range(3):
        W = [cW[:, :, ty, dxi].T for ty in range(3)]  # [128, 64] each
        cwt[:, dxi * 4 + 0, 0], cwt[:, dxi * 4 + 0, 1] = W[1], W[2]
        cwt[:, dxi * 4 + 1, 0], cwt[:, dxi * 4 + 1, 1] = W[0], W[1]
        cwt[:, dxi * 4 + 2, 0] = W[0]
        cwt[:, dxi * 4 + 3, 0] = W[2]
    negu16 = np.zeros((16, 2048), np.float32)
    onesel = np.zeros((16, 2048), np.float32)
    for j in range(16):
        negu16[j, 128 * j:128 * j + 128] = -u
        onesel[j, 128 * j:128 * j + 128] = 1.0
    return {
        "Amat": np.ascontiguousarray(A.astype(NPBF16)),
        "stx": np.ascontiguousarray(stx),
        "negu": np.ascontiguousarray(negu16.astype(NPBF16)),
        "onesel": np.ascontiguousarray(onesel.astype(NPBF16)),
        "ipair": np.ascontiguousarray(ipair),
        "cbias": np.ascontiguousarray(c_b.reshape(128, 1)),
        "vwf": np.ascontiguousarray(vWf.astype(NPBF16)),
        "vbp": np.ascontiguousarray(vbp.reshape(128, 1)),
        "cwt": np.ascontiguousarray(cwt.reshape(128, 1536).astype(NPF8)),
        "cbb": np.ascontiguousarray(cb.reshape(64, 1)),
        "vsel": np.ascontiguousarray(vsel),
        "vsel2": np.ascontiguousarray(vsel2.astype(NPBF16)),
        "negw1": np.ascontiguousarray(negw1.astype(NPBF16)),
    }


def _run(in_maps, trace=False, **kw):
    if "nc" not in _CACHE:
        _CACHE["nc"] = _build_nc()
    return run_bass_kernel_spmd(_CACHE["nc"], in_maps, list(range(8)),
                                trace=trace, **kw)


def kernel(q, v, qW, qb, vW, vb, K, qn_g, qn_b, vn_g, vn_b, cW, cb):
    base = _fold_weights(qW, qb, vW, vb, K, qn_g, qn_b, vn_g, vn_b, cW, cb)
    in_maps = []
    for i in range(8):
        m = dict(base)
        m["q"] = np.ascontiguousarray(np.float32(q[i]).reshape(64, 65536))
        m["v"] = np.ascontiguousarray(np.float32(v[i]).reshape(128, 4096))
        in_maps.append(m)
    res = _run(in_maps)
    outs = [np.asarray(r["out"]).astype(np.float32).reshape(64, 256, 256)
            for r in res.results]
    return np.stack(outs)


# revision 35
# speedup vs baseline: 3.1940x; 1.0394x over previous
"""LocalPatchAttention Trainium2 kernel.

Data-parallel over batch B=8 across 8 NeuronCores (one image per core).
Per-core pipeline for B,Cq,H,W = 8,64,256,256 / Cv,h,w = 128,64,64.

Transpose-free LayerNorm-attention formulation:
  logits[v,px] = r_px * (A.T q[:,px] - mu_px * u) + cb
with A = scale*(g*qW.T)@K.T prefolded on host, u = colsum(A),
mu/E[q^2] per pixel computed by PE ones-matmuls on float32r views of the
raw f32 q rows (1 cycle/row, no bf16 copy), and the per-pixel row math
(var, 1/sqrt) batched over 16 row-pairs so its DVE/Act cost amortizes.
r is broadcast across the 128 v-channels by a rank-1 ones-matmul; the
single DVE multiply X = lg * R feeds Sigmoid (bias = folded cb).
x_attn = sig * V uses a pre-replicated bf16 V (V_rep) so the multiply
runs in the DVE fast mode. 3x3 conv = 12 PSUM-accumulated bf16 matmuls
per 4 output rows (2-row-deep output packing), conv bias as a K=1
matmul, and the residual q added by two identity matmuls per row pair
(float32r). conv PSUM is copied once to bf16 SBUF and DMAed out as
bf16 (upcast on host).

Activation usage stays inside {Square, Copy, Sigmoid} plus one Sqrt per
16-pair batch, so act-table reloads drop from ~2/pair to 2/batch.
"""

import numpy as np
import ml_dtypes

import concourse.bass as bass
import concourse.bacc as bacc
import concourse.tile as tile
from concourse import mybir
from concourse.bass_utils import run_bass_kernel_spmd

F32 = mybir.dt.float32
F32R = mybir.dt.float32r
F8E4 = mybir.dt.float8e4
BF16 = mybir.dt.bfloat16
AF = mybir.ActivationFunctionType
ALU = mybir.AluOpType
EPS = 1e-5
NPBF16 = ml_dtypes.bfloat16
NPF8 = ml_dtypes.float8_e4m3

_CACHE = {}


def _build_nc():
    nc = bacc.Bacc()
    q_d = nc.declare_dram_parameter("q", [64, 65536], F32R, isOutput=False)
    v_d = nc.declare_dram_parameter("v", [128, 4096], BF16, isOutput=False)
    A_d = nc.declare_dram_parameter("Amat", [64, 128], BF16, isOutput=False)
    stx_d = nc.declare_dram_parameter("stx", [128, 768], F32R, isOutput=False)
    negu_d = nc.declare_dram_parameter("negu", [16, 2048], BF16, isOutput=False)
    onesel_d = nc.declare_dram_parameter("onesel", [16, 2048], BF16, isOutput=False)
    ip_d = nc.declare_dram_parameter("ipair", [64, 64], F32R, isOutput=False)
    cb_d = nc.declare_dram_parameter("cbias", [128, 1], F32, isOutput=False)
    vwf_d = nc.declare_dram_parameter("vwf", [128, 128], BF16, isOutput=False)
    vbp_d = nc.declare_dram_parameter("vbp", [128, 1], F32, isOutput=False)
    cwt_d = nc.declare_dram_parameter("cwt", [128, 1536], F8E4, isOutput=False)
    cbb_d = nc.declare_dram_parameter("cbb", [64, 1], F32, isOutput=False)
    vsel_d = nc.declare_dram_parameter("vsel", [128, 640], BF16, isOutput=False)
    vsel2_d = nc.declare_dram_parameter("vsel2", [8, 1024], BF16, isOutput=False)
    negw1_d = nc.declare_dram_parameter("negw1", [8, 1024], BF16, isOutput=False)
    out_d = nc.declare_dram_parameter("out", [64, 65536], BF16, isOutput=True)

    with tile.TileContext(nc) as tc, \
         tc.tile_pool(name="const", bufs=1) as cpool, \
         tc.tile_pool(name="vwork", bufs=1) as vpool, \
         tc.tile_pool(name="qq", bufs=11) as qq_pool, \
         tc.tile_pool(name="stat", bufs=2) as st_pool, \
         tc.tile_pool(name="xt", bufs=3) as x_pool, \
         tc.tile_pool(name="sig", bufs=3) as sig_pool, \
         tc.tile_pool(name="srow", bufs=12) as srow_pool, \
         tc.tile_pool(name="outp", bufs=3) as out_pool, \
         tc.tile_pool(name="ps_s", bufs=2, space="PSUM") as ps_s, \
         tc.tile_pool(name="ps_lg", bufs=2, space="PSUM") as ps_lg, \
         tc.tile_pool(name="ps_r", bufs=2, space="PSUM") as ps_r, \
         tc.tile_pool(name="ps_cv", bufs=2, space="PSUM") as ps_cv:

        def const_tile(shape, dtype, tag, src):
            t = cpool.tile(shape, dtype, tag=tag)
            nc.sync.dma_start(out=t, in_=src[:, :])
            return t

        A_sb = const_tile([64, 128], BF16, "A", A_d)
        stx_sb = const_tile([128, 768], F32R, "stx", stx_d)
        negu_sb = const_tile([16, 2048], BF16, "negu", negu_d)
        onesel_sb = const_tile([16, 2048], BF16, "onesel", onesel_d)
        ip_sb = const_tile([64, 64], F32R, "ip", ip_d)
        cb_sb = const_tile([128, 1], F32, "cb", cb_d)
        vwf_sb = const_tile([128, 128], BF16, "vwf", vwf_d)
        vbp_sb = const_tile([128, 1], F32, "vbp", vbp_d)
        cwt_sb = const_tile([128, 1536], F8E4, "cwt", cwt_d)
        cbb_sb = const_tile([64, 1], F32, "cbb", cbb_d)

        ones1x128b = cpool.tile([1, 128], BF16, tag="o1x")
        nc.vector.memset(ones1x128b, 1.0)
        ones128b = cpool.tile([128, 1], BF16, tag="o128")
        nc.vector.memset(ones128b, 1.0)
        vsel_sb = const_tile([128, 640], BF16, "vsel", vsel_d)
        vsel2_sb = const_tile([8, 1024], BF16, "vsel2", vsel2_d)
        negw1_sb = const_tile([8, 1024], BF16, "negw1", negw1_d)
        epsc = cpool.tile([128, 1], F32, tag="eps")
        nc.vector.memset(epsc, EPS)

        # ---------------- V path (once per core) ----------------
        # batched stats: vst[0:8]=col-sums/128 per chunk, vst[32:40]=sq-sums/128
        vraw = vpool.tile([128, 4096], BF16, tag="vraw")
        vsqr = vpool.tile([128, 4096], BF16, tag="vsqr")
        V_sbs = {}
        for ch in range(8):
            V_sbs[ch] = vpool.tile([128, 512], BF16, tag=f"V{ch}", name=f"V{ch}")
        vst = ps_lg.tile([40, 512], F32, tag="lg", name="vst")
        for ch in range(8):
            sl = slice(ch * 512, (ch + 1) * 512)
            nc.sync.dma_start(out=vraw[:, sl], in_=v_d[:, sl])
            nc.gpsimd.tensor_mul(vsqr[:, sl], vraw[:, sl], vraw[:, sl])
            nc.tensor.matmul(vst, vsel_sb[:, 80 * ch:80 * ch + 40], vraw[:, sl],
                             start=(ch == 0), stop=False)
            nc.tensor.matmul(vst, vsel_sb[:, 80 * ch + 40:80 * ch + 80], vsqr[:, sl],
                             start=False, stop=(ch == 7))
        m2v = st_pool.tile([8, 512], F32, tag="m2v")
        nc.scalar.activation(m2v, vst[0:8, :], AF.Square)
        varv = st_pool.tile([8, 512], F32, tag="varv")
        nc.vector.tensor_sub(varv, vst[32:40, :], m2v)
        sdv = st_pool.tile([8, 512], F32, tag="sdv")
        nc.scalar.activation(sdv, varv, AF.Sqrt, bias=epsc[0:8, 0:1])
        rrv = st_pool.tile([8, 512], BF16, tag="rrv")
        with nc.allow_low_precision(reason="bf16 LN rstd; tol 2e-2"):
            nc.vector.reciprocal(rrv, sdv)
        rmv = st_pool.tile([8, 512], BF16, tag="rmv")
        nc.vector.tensor_mul(rmv, vst[0:8, :], rrv)
        for ch in range(8):
            sl = slice(ch * 512, (ch + 1) * 512)
            rb = ps_lg.tile([128, 512], F32, tag="lg", name="rb")
            nc.tensor.matmul(rb, vsel2_sb[:, 128 * ch:128 * ch + 128], rrv,
                             start=True, stop=True)
            vt = x_pool.tile([128, 512], BF16, tag="vt")
            nc.vector.tensor_mul(vt, vraw[:, sl], rb)
            vp = ps_cv.tile([128, 512], F32, tag="cv", name="vp")
            nc.tensor.matmul(vp, vwf_sb, vt, start=True, stop=False)
            nc.tensor.matmul(vp, negw1_sb[:, 128 * ch:128 * ch + 128], rmv,
                             start=False, stop=True)
            nc.vector.tensor_scalar_add(V_sbs[ch], vp, vbp_sb[:, 0:1])

        # ---------------- main loop ----------------
        qqs = {}     # quad index k -> [128,1024] tile (rows 4k..4k+3; q | q^2)
        srows = {}   # pair index -> [128,512] bf16 tile (rows 2i, 2i+1)
        s32s = {}    # batch -> [48,512] psum stats (mu rows 0:16, msq 32:48)
        r16s = {}
        rm16s = {}

        BATCHES = [(0, 4), (4, 12)] + [(16 * b, 16) for b in range(1, 8)]
        PAIR_BATCH = {}
        for bi_, (bs_, sz_) in enumerate(BATCHES):
            for jj_ in range(sz_):
                PAIR_BATCH[bs_ + jj_] = (bi_, bs_, jj_)

        def phase1(i):
            b, bs, j = PAIR_BATCH[i]
            sz = BATCHES[b][1]
            if i % 2 == 0:
                k = i // 2
                qq = qq_pool.tile([128, 1024], F32R, tag="qq")
                nc.sync.dma_start(out=qq[0:64, :], in_=q_d[:, k * 1024:(k + 1) * 1024])
                if k < 4:
                    nc.scalar.activation(qq[64:128, :], qq[0:64, :], AF.Square)
                else:
                    nc.gpsimd.tensor_mul(qq[64:128, :], qq[0:64, :], qq[0:64, :])
                qqs[k] = qq
            if j == 0:
                s32s[b] = ps_s.tile([48, 512], F32, tag="s32", name="s32")
            qq = qqs[i // 2]
            sl = slice((i % 2) * 512, (i % 2) * 512 + 512)
            nc.tensor.matmul(s32s[b], stx_sb[:, 48 * j:48 * j + 48],
                             qq[:, sl], start=(j == 0), stop=(j == sz - 1))

        def rowmath(b):
            s32 = s32s.pop(b)
            m2 = st_pool.tile([16, 512], F32, tag="m2")
            nc.scalar.activation(m2, s32[0:16, :], AF.Square)
            varp = st_pool.tile([16, 512], F32, tag="varp")
            nc.vector.tensor_sub(varp, s32[32:48, :], m2)
            sd = st_pool.tile([16, 512], F32, tag="sd")
            nc.scalar.activation(sd, varp, AF.Sqrt, bias=epsc[0:16, 0:1])
            r16 = st_pool.tile([16, 512], BF16, tag="r16")
            with nc.allow_low_precision(reason="bf16 r feeds sigmoid logits; tol 2e-2"):
                nc.vector.reciprocal(r16, sd)
            rm16 = st_pool.tile([16, 512], BF16, tag="rm16")
            nc.vector.tensor_mul(rm16, s32[0:16, :], r16)
            r16s[b], rm16s[b] = r16, rm16

        def phase2(i):
            b, bs, j = PAIR_BATCH[i]
            qq = qqs[i // 2]
            sl = slice((i % 2) * 512, (i % 2) * 512 + 512)
            R = ps_r.tile([128, 512], F32, tag="R")
            nc.tensor.matmul(R, onesel_sb[:, 128 * j:128 * j + 128], r16s[b],
                             start=True, stop=True)
            qtil = x_pool.tile([64, 512], BF16, tag="qtil")
            nc.vector.tensor_mul(qtil, qq[0:64, sl].bitcast(F32), R[0:64, :])
            lg = ps_lg.tile([128, 512], F32, tag="lg")
            nc.tensor.matmul(lg, A_sb, qtil, start=True, stop=False)
            nc.tensor.matmul(lg, negu_sb[:, 128 * j:128 * j + 128], rm16s[b],
                             start=False, stop=True)
            sig = sig_pool.tile([128, 512], BF16, tag="sig")
            nc.scalar.activation(sig, lg, AF.Sigmoid, bias=cb_sb[:, 0:1])
            hy = i // 2
            vr = V_sbs[hy // 8][:, (hy % 8) * 64:(hy % 8) * 64 + 64] \
                .rearrange("p c -> p () c ()").broadcast_to([128, 2, 64, 4])
            srow = srow_pool.tile([128, 512], F8E4, tag="srow")
            with nc.allow_low_precision(reason="fp8 conv input; fp8 conv sim err 7e-3, tol 2e-2"):
                nc.vector.tensor_mul(
                    srow.rearrange("p (s c f) -> p s c f", s=2, c=64, f=4),
                    sig.rearrange("p (s c f) -> p s c f", s=2, c=64, f=4), vr)
            srows[i] = srow

        DR = mybir.MatmulPerfMode.DoubleRow

        def conv_block(i):
            # rows r0=2i (even), r1=2i+1 (odd); unpacked out [64ch, 2x256]
            cv = ps_cv.tile([64, 512], F32, tag="cv")
            sl = slice((i % 2) * 512, (i % 2) * 512 + 512)
            qq = qqs[i // 2]
            nc.tensor.matmul(cv, ip_sb, qq[0:64, sl], start=True, stop=False)
            mms = []
            sp = srows[i].rearrange("p (s x) -> p s x", s=2)
            for dxi in range(3):
                lo, olo, n = ((0, 1, 255), (0, 0, 256), (1, 0, 255))[dxi]
                for rpar in range(2):
                    base = rpar * 256
                    out = cv[:, base + olo:base + olo + n]
                    wp = cwt_sb[:, (dxi * 4 + rpar) * 128:(dxi * 4 + rpar + 1) * 128] \
                        .rearrange("p (s m) -> p s m", s=2)
                    mms.append((out, wp, sp[:, :, lo:lo + n]))
                    if rpar == 0 and i > 0:
                        rs = srows[i - 1][:, 256 + lo:256 + lo + n] \
                            .rearrange("p x -> p () x").broadcast_to([128, 2, n])
                    elif rpar == 1 and i < 127:
                        rs = srows[i + 1][:, lo:lo + n] \
                            .rearrange("p x -> p () x").broadcast_to([128, 2, n])
                    else:
                        continue
                    ws = cwt_sb[:, (dxi * 4 + 2 + rpar) * 128:(dxi * 4 + 3 + rpar) * 128] \
                        .rearrange("p (s m) -> p s m", s=2)
                    mms.append((out, ws, rs))
            for mi, (out, w, rhs) in enumerate(mms):
                nc.tensor.matmul(out, w, rhs, start=False, stop=(mi == len(mms) - 1),
                                 perf_mode=DR)
            if i % 2 == 1:
                qqs.pop(i // 2)
            ot = out_pool.tile([64, 512], BF16, tag="ot")
            nc.scalar.activation(ot, cv, AF.Identity, bias=cbb_sb[:, 0:1])
            nc.sync.dma_start(out=out_d[:, 2 * i * 256:(2 * i + 2) * 256], in_=ot)
            for r in list(srows):
                if r < i - 1:
                    del srows[r]

        ROWMATH_STEP = {bs + sz - 1: bi for bi, (bs, sz) in enumerate(BATCHES)}
        P2_STEP = {}
        for bi_, (bs_, sz_) in enumerate(BATCHES):
            for jj_ in range(sz_):
                P2_STEP.setdefault(bs_ + sz_ + jj_, []).append(bs_ + jj_)
        p2_done = -1
        conv_done = -1
        for step in range(146):
            if step < 128:
                phase1(step)
            if step in ROWMATH_STEP:
                rowmath(ROWMATH_STEP[step])
            for p2 in P2_STEP.get(step, []):
                phase2(p2)
                p2_done = p2
            while conv_done + 1 <= 127 and (p2_done >= conv_done + 2
                                            or p2_done == 127):
                conv_done += 1
                conv_block(conv_done)

    nc.finalize()
    return nc


def _fold_weights(qW, qb, vW, vb, K, qn_g, qn_b, vn_g, vn_b, cW, cb):
    f = np.float32
    qW, qb, vW, vb, K = f(qW), f(qb), f(vW), f(vb), f(K)
    qn_g, qn_b, vn_g, vn_b, cW, cb = f(qn_g), f(qn_b), f(vn_g), f(vn_b), f(cW), f(cb)
    scale = np.float32(64.0 ** -0.5)
    qWf = qn_g[:, None] * qW.T                      # [c, co]
    bprime = qb + qW @ qn_b                         # [64]
    A = scale * (qWf @ K.T)                         # [64, 128]
    c_b = scale * (K @ bprime)                      # [128]
    u = A.sum(axis=0)                               # [128]
    stx = np.zeros((128, 768), np.float32)
    for i in range(16):
        stx[0:64, 48 * i + i] = 1.0 / 64
        stx[64:128, 48 * i + 32 + i] = 1.0 / 64
    ipair = np.eye(64, dtype=np.float32)
    vWf = vn_g[:, None] * vW.T                      # [128, 128]
    vbp = vb + vW @ vn_b                            # [128]
    w1 = vWf.sum(axis=0)                            # [128]
    vsel = np.zeros((128, 640), np.float32)
    vsel2 = np.zeros((8, 1024), np.float32)
    negw1 = np.zeros((8, 1024), np.float32)
    for j in range(8):
        vsel[:, 80 * j + j] = 1.0 / 128
        vsel[:, 80 * j + 40 + 32 + j] = 1.0 / 128
        vsel2[j, 128 * j:128 * j + 128] = 1.0
        negw1[j, 128 * j:128 * j + 128] = -w1
    # fp8 DoubleRow conv weights: blk = dxi*4 + kind, each [2, 64] (s, m)
    # kind 0 pair-even (ty1, ty2); 1 pair-odd (ty0, ty1);
    # kind 2 single-even (ty0, 0); 3 single-odd (ty2, 0)
    cwt = np.zeros((128, 12, 2, 64), np.float32)
    for dxi in range(3):
        W = [cW[:, :, ty, dxi].T for ty in range(3)]  # [128, 64] each
        cwt[:, dxi * 4 + 0, 0], cwt[:, dxi * 4 + 0, 1] = W[1], W[2]
        cwt[:, dxi * 4 + 1, 0], cwt[:, dxi * 4 + 1, 1] = W[0], W[1]
        cwt[:, dxi * 4 + 2, 0] = W[0]
        cwt[:, dxi * 4 + 3, 0] = W[2]
    negu16 = np.zeros((16, 2048), np.float32)
    onesel = np.zeros((16, 2048), np.float32)
    for j in range(16):
        negu16[j, 128 * j:128 * j + 128] = -u
        onesel[j, 128 * j:128 * j + 128] = 1.0
    return {
        "Amat": np.ascontiguousarray(A.astype(NPBF16)),
        "stx": np.ascontiguousarray(stx),
        "negu": np.ascontiguousarray(negu16.astype(NPBF16)),
        "onesel": np.ascontiguousarray(onesel.astype(NPBF16)),
        "ipair": np.ascontiguousarray(ipair),
        "cbias": np.ascontiguousarray(c_b.reshape(128, 1)),
        "vwf": np.ascontiguousarray(vWf.astype(NPBF16)),
        "vbp": np.ascontiguousarray(vbp.reshape(128, 1)),
        "cwt": np.ascontiguousarray(cwt.reshape(128, 1536).astype(NPF8)),
        "cbb": np.ascontiguousarray(cb.reshape(64, 1)),
        "vsel": np.ascontiguousarray(vsel.astype(NPBF16)),
        "vsel2": np.ascontiguousarray(vsel2.astype(NPBF16)),
        "negw1": np.ascontiguousarray(negw1.astype(NPBF16)),
    }


def _run(in_maps, trace=False, **kw):
    if "nc" not in _CACHE:
        _CACHE["nc"] = _build_nc()
    return run_bass_kernel_spmd(_CACHE["nc"], in_maps, list(range(8)),
                                trace=trace, **kw)


def kernel(q, v, qW, qb, vW, vb, K, qn_g, qn_b, vn_g, vn_b, cW, cb):
    base = _fold_weights(qW, qb, vW, vb, K, qn_g, qn_b, vn_g, vn_b, cW, cb)
    in_maps = []
    for i in range(8):
        m = dict(base)
        m["q"] = np.ascontiguousarray(np.float32(q[i]).reshape(64, 65536))
        m["v"] = np.ascontiguousarray(np.float32(v[i]).reshape(128, 4096).astype(NPBF16))
        in_maps.append(m)
    res = _run(in_maps)
    outs = [np.asarray(r["out"]).astype(np.float32).reshape(64, 256, 256)
            for r in res.results]
    return np.stack(outs)
